# revision 50
# baseline (speedup 1.0000x reference)
"""GAT (2-layer PyG GATConv, eval) on 8 Trainium2 NeuronCores.

Sharding: nodes range-partitioned (NLOC=12800/core); core c owns edges whose
dst is in its range. Both layers' node tables are computed SHARDED (each core
transforms only its own 12800-node block) and replicated by one AllGather per
layer; each AllGather is fully overlapped by the a_dst gather burst for the
next edge pass, which reads only the local block.

Slot layout per core: superblock (10 windows) -> quadrant -> window, with
per-(window,quadrant) STATIC capacities = max edge count over the 8 cores
(SPMD: one module runs on all cores; only tensor contents differ) — ~10%
slot padding vs 28% for fixed-size groups. Gather calls use
single_packet=False, which lets the SWDGE ucode stream descriptors through
the ring: up to 2x the ring (dynamic_dma_scratch_size/16) indices per call
(probed on HW; single-packet calls hard-crash above 1024 idx). 4096-idx
calls cut the 994ns-per-call Pool overhead 4x vs the 1024-idx baseline.

Per layer, per edge slot: a 144B/82B payload gather pulls [h|a_src] rows
(256B-stride tables, int16 idx into 25600-row quadrants). a_dst is NOT
gathered: slots are drel-sorted inside each (window,quadrant) group, so
per-slot a_dst[drel] telescopes — P[d,s] = (s >= start[d]) is a DVE is_ge
step matrix (host-static start tables) and PE computes P^T @ diff(a_dst)
per chunk column; window starts mid-chunk use a compensated diff table
(row0 = ad[0,w]-ad[127,w-1]) accumulated in the same PSUM. Both at1 and at2
expansions run during their layer's AllGather (they read only local data).
Softmax is the shift-invariant no-max form (w = exp(leakyrelu(as+ad)),
|e| < ~25 so fp32 exp is safe). Segment reduction is a PE matmul whose
stationary matrix is a transposed one-hot built by DVE is_equal in the
2x-mode layout; boundary chunks carry a masked drel column per touching
window. msg = h*w uses an Act-expanded weight tile so the DVE mult runs in
2x mode. Softmax normalize + ELU + the r2 = hlT @ [W2|a2_src|a2_dst]
projection are batched per superblock; tables are written node-permuted
(row = p*100 + j within each core block) so writes coalesce per partition.
"""
import numpy as np
import ml_dtypes

N = 100000
E = 1600000
NF = 256
HEADS, NHID = 8, 8
NH = HEADS * NHID          # 64
NCLASS = 40
NLOC = 12800               # nodes per core
NW = 100                   # 128-dst windows per core
NQ = 4                     # src table quadrants
QS = 25600                 # rows per quadrant
NWSB = 10                  # windows per superblock
NSB = NW // NWSB           # 10 superblocks
NTOT = 102400
ACC_EPS = 1e-16

_CACHE = {}


def _ceil128(x):
    return (x + 127) & ~127


def _host_prep(x, edge_index, W1, a1_src, a1_dst, b1, W2, a2_src, a2_dst, b2):
    src = np.asarray(edge_index[0], dtype=np.int64)
    dst = np.asarray(edge_index[1], dtype=np.int64)

    # table-row permutation: node n -> row  c*NLOC + (l%128)*NW + l//128
    def rowperm(n):
        c = n // NLOC
        l = n - c * NLOC
        return c * NLOC + (l % 128) * NW + l // 128

    srow = rowperm(src)
    sq = srow // QS
    sidx = (srow - sq * QS).astype(np.int16)

    core = dst // NLOC
    dloc = dst - core * NLOC
    w_e = (dloc >> 7).astype(np.int64)
    dr_e = (dloc & 127).astype(np.int64)
    adidx = (dr_e * NW + w_e).astype(np.int16)
    sb_e = w_e // NWSB

    # static capacities: max over cores per (window, quadrant); >=1 so every
    # group is present in the slot stream (the telescoped a_dst expansion
    # needs window w-1 to precede window w inside each (sb,q) segment)
    gkey = (core * NW + w_e) * NQ + sq          # [E]
    cnt = np.bincount(gkey, minlength=8 * NW * NQ).reshape(8, NW, NQ)
    cap = np.maximum(cnt.max(axis=0), 1)         # [NW, NQ]

    # slot layout: sb -> quadrant -> window.  With single_packet=False the
    # SWDGE gather ucode streams descriptors through the ring, so calls up
    # to 2x the ring size (dynamic_dma_scratch_size/16) are fine (probed on
    # HW: 8192 idx with a 4096-desc ring).
    MAXIDX = 8192
    wq_start = np.zeros((NW, NQ), np.int64)
    hcalls = []                                  # (sb, q, slot0, n_idx)
    sb_chunks = []                               # (k0, k1) per sb
    nslot = 0
    for s in range(NSB):
        k0 = nslot // 128
        for q in range(NQ):
            seg0 = nslot
            for w in range(s * NWSB, (s + 1) * NWSB):
                wq_start[w, q] = nslot
                nslot += int(cap[w, q])
            nslot = _ceil128(nslot)
            for off in range(seg0, nslot, MAXIDX):
                hcalls.append((s, q, off, min(MAXIDX, nslot - off)))
        sb_chunks.append((k0, nslot // 128))
    NSLOT = nslot
    NCHUNK = NSLOT // 128

    # window label per slot (shared across cores: layout is static).  Group
    # pads inherit their group's window; (sb,q)-tail ceil128 pads inherit the
    # segment's last window.
    wfull = np.zeros(NSLOT, np.int64)
    for s in range(NSB):
        for q in range(NQ):
            for w in range(s * NWSB, (s + 1) * NWSB):
                a = int(wq_start[w, q])
                e_ = a + int(cap[w, q])
                wfull[a:e_] = w
            wfull[e_:_ceil128(e_)] = (s + 1) * NWSB - 1   # segment tail pads

    # at-plan: per chunk, one column-copy per touching window; copies of one
    # chunk accumulate in PSUM.  v=1 (compensated diff table) iff the window
    # starts mid-chunk.  Packed into segments of <= ATSEGC columns.
    ATSEGC = 24
    atsegs = []            # (kg0, nk, c0, cols=[(i, klocal, v, w, first, last)])
    copies_per_chunk = []  # [(w, a, b)]
    for k in range(NCHUNK):
        wk = wfull[k * 128:(k + 1) * 128]
        runs = []
        a = 0
        for p in range(1, 128):
            if wk[p] != wk[p - 1]:
                runs.append((int(wk[a]), a, p))
                a = p
        runs.append((int(wk[a]), a, 128))
        copies_per_chunk.append(runs)
    ncolat = 0
    cur = None
    colat_of = {}
    for k in range(NCHUNK):
        runs = copies_per_chunk[k]
        if cur is None or cur[3] + len(runs) > ATSEGC:
            if cur is not None:
                atsegs.append(cur)
            cur = [k, 0, ncolat, 0, []]
        kloc = cur[1]
        for i, (w, a, b) in enumerate(runs):
            colat_of[(k, a)] = ncolat
            cur[4].append((cur[3], kloc, 1 if a > 0 else 0, w,
                           i == 0, i == len(runs) - 1))
            cur[3] += 1
            ncolat += 1
        cur[1] += 1
    if cur is not None:
        atsegs.append(cur)
    NCOLAT = ncolat

    # per-window chunk columns
    colmap = np.full((NW, NCHUNK), -1, np.int64)
    wcols = []                                   # per w: (colbase, [chunks])
    ncol = 0
    for w in range(NW):
        cols = []
        for q in range(NQ):
            a = int(wq_start[w, q])
            b = a + int(cap[w, q])
            for k in range(a // 128, (b + 127) // 128):
                cols.append(k)
                colmap[w, k] = ncol
                ncol += 1
        wcols.append(cols)
    NCOL = ncol
    MAXCPW = max(len(c) for c in wcols)

    plan = {
        "NSLOT": NSLOT, "NCHUNK": NCHUNK, "NCOL": NCOL, "MAXCPW": MAXCPW,
        "NCOLAT": NCOLAT, "ATSEGC": ATSEGC,
        "hcalls": hcalls, "atsegs": atsegs, "sb_chunks": sb_chunks,
        "wcols": wcols,
        "skip_b1": bool(np.all(np.asarray(b1) == 0)),
        "skip_b2": bool(np.all(np.asarray(b2) == 0)),
    }

    # group-id in slot order: (sb, q, w_in_sb)
    flatg = (sb_e * NQ + sq) * NWSB + (w_e - sb_e * NWSB)
    gstart_flat = np.zeros(NSB * NQ * NWSB, np.int64)
    for s in range(NSB):
        for q in range(NQ):
            for wi in range(NWSB):
                gstart_flat[(s * NQ + q) * NWSB + wi] = wq_start[s * NWSB + wi, q]

    per_core = []
    hidx_all, startd_all, drel_all = [], [], []
    for c in range(8):
        m = core == c
        fg = flatg[m]
        drc = dr_e[m]
        order = np.lexsort((drc, fg))
        fgs = fg[order]
        cntc = np.bincount(fgs, minlength=NSB * NQ * NWSB)
        starts = np.zeros_like(cntc)
        starts[1:] = np.cumsum(cntc)[:-1]
        rank = np.arange(len(fgs)) - starts[fgs]
        slot = gstart_flat[fgs] + rank

        hvec = np.zeros(NSLOT, np.int16)
        hvec[slot] = sidx[m][order]

        drel = np.full((128, NCOL), 128.0, np.float32)
        k_s = slot >> 7
        p_s = slot & 127
        we_s = w_e[m][order]
        col_s = colmap[we_s, k_s]
        assert (col_s >= 0).all()
        drel[p_s, col_s] = drc[order].astype(np.float32)

        # per-slot drel stream (pads = 128) for the telescoped start tables
        drfull = np.full(NSLOT, 128, np.int64)
        drfull[slot] = drc[order]
        startd = np.zeros((128, NCOLAT), np.float32)
        dgrid = np.arange(128)
        for k in range(NCHUNK):
            for (w, a, b) in copies_per_chunk[k]:
                col = colat_of[(k, a)]
                drs = drfull[k * 128 + a:k * 128 + b]
                startd[:, col] = a + np.searchsorted(drs, dgrid)

        def wrap16(v):
            o = np.zeros((128, NSLOT // 16), np.int16)
            sl = np.arange(NSLOT)
            o[sl % 16, sl // 16] = v
            for r in range(1, 8):
                o[16 * r:16 * (r + 1)] = o[:16]
            return o

        hidx_all.append(wrap16(hvec))
        startd_all.append(startd.astype(np.float16))
        drel_all.append(drel.astype(ml_dtypes.bfloat16))

    # weights
    W1 = np.asarray(W1, np.float32)
    v_s1 = np.einsum("chk,hk->ch", W1.reshape(NF, HEADS, NHID),
                     np.asarray(a1_src, np.float32))
    v_d1 = np.einsum("chk,hk->ch", W1.reshape(NF, HEADS, NHID),
                     np.asarray(a1_dst, np.float32))
    W1e = np.concatenate([W1, v_s1, v_d1], axis=1).reshape(2, 128, 80)
    W1e = W1e.astype(np.float16)

    W2 = np.asarray(W2, np.float32)
    v_s2 = W2 @ np.asarray(a2_src, np.float32)[0]
    v_d2 = W2 @ np.asarray(a2_dst, np.float32)[0]
    W2e = np.concatenate([W2, v_s2[:, None], v_d2[:, None]],
                         axis=1).astype(np.float16)   # [64, 42]

    xp = np.zeros((NTOT, NF), np.float32)
    xp[:N] = np.asarray(x, np.float32)

    # lhsT matrices for building the diff tables on PE:
    # mshift[:,0,:] = Mplain^T (fwd diff), mshift[:,1,:] = -sel(127)->row0
    mshiftT = np.zeros((128, 2, 128), np.float16)
    mshiftT[:, 0, :] = (np.eye(128) - np.eye(128, k=1)).astype(np.float16)
    mshiftT[127, 1, 0] = -1.0

    for c in range(8):
        xloc = np.ascontiguousarray(xp[c * NLOC:(c + 1) * NLOC].T)
        per_core.append({
            "xTloc": xloc.astype(np.float16).reshape(2, 128, NLOC),
            "W1e": W1e,
            "W2e": W2e,
            "b1": np.asarray(b1, np.float32)[None, :],
            "b2": np.asarray(b2, np.float32)[None, :],
            "hidx": hidx_all[c],
            "startd": startd_all[c],
            "drel": drel_all[c],
            "mshift": mshiftT,
        })
    return per_core, plan


def _gather_small(g, out_ap, in_ap, idxs_ap, num_idxs, elem_size, elem_step,
                  queue_num=0, single_packet=False):
    """dma_gather with payload < 256B; only the 256B row-stride rule is real
    for the non-transpose path."""
    import concourse.mybir as mybir
    stride_bytes = elem_step * mybir.dt.size(in_ap.dtype)
    assert stride_bytes % 256 == 0
    _in_ap = g.lower_ap_dma(in_ap, for_custom_bir_dma=True)
    _idxs_ap = g.lower_ap(idxs_ap)
    _out_ap = g.lower_ap(out_ap)
    return g.add_instruction(mybir.InstDMAGatherAnt(
        name=g.bass.get_next_instruction_name(),
        ins=[*_in_ap, _idxs_ap, g.lower_val_access(g.to_reg(num_idxs))],
        outs=[_out_ap],
        transpose=False,
        num_idxs=num_idxs,
        elem_size=elem_size,
        stride_bytes_256=stride_bytes // 256,
        gen_mode=0,
        single_packet=single_packet,
        queue_num=queue_num,
        sbuf_tokens_per_rank=0,
        sbuf_free_dim_per_rank=0,
        sbuf_free_dim_pad_per_rank=0,
        sbuf_byte_offset=0,
    ))


def _build_nc(plan):
    import concourse.bass as bass
    import concourse.bacc as bacc
    import concourse.mybir as mybir
    import concourse.tile as tile
    from concourse.library_config import mlp
    from concourse.masks import make_identity

    f32, f16, bf16, i16 = (mybir.dt.float32, mybir.dt.float16,
                           mybir.dt.bfloat16, mybir.dt.int16)
    AF = mybir.ActivationFunctionType
    OP = mybir.AluOpType

    NSLOT = plan["NSLOT"]
    NCOL = plan["NCOL"]
    MAXCPW = plan["MAXCPW"]
    sb_chunks = plan["sb_chunks"]
    wcols = plan["wcols"]
    CPSB_MAX = max(k1 - k0 for k0, k1 in sb_chunks)
    colbase = [0] * NW
    for w in range(1, NW):
        colbase[w] = colbase[w - 1] + len(wcols[w - 1])

    NCOLAT = plan["NCOLAT"]
    ATSEGC = plan["ATSEGC"]
    atsegs = plan["atsegs"]

    nc = bacc.Bacc("TRN2", target_bir_lowering=False, debug=False,
                   num_devices=8, num_swdge_queues=4,
                   dynamic_dma_scratch_size=32768)

    xTloc = nc.dram_tensor("xTloc", [2, 128, NLOC], f16, kind="ExternalInput")
    W1e = nc.dram_tensor("W1e", [2, 128, 80], f16, kind="ExternalInput")
    W2e = nc.dram_tensor("W2e", [64, 42], f16, kind="ExternalInput")
    b1 = nc.dram_tensor("b1", [1, 64], f32, kind="ExternalInput")
    b2 = nc.dram_tensor("b2", [1, 40], f32, kind="ExternalInput")
    hidx_d = nc.dram_tensor("hidx", [128, NSLOT // 16], i16,
                            kind="ExternalInput")
    startd_d = nc.dram_tensor("startd", [128, NCOLAT], f16,
                              kind="ExternalInput")
    mshift_d = nc.dram_tensor("mshift", [128, 2, 128], f16,
                              kind="ExternalInput")
    drel_d = nc.dram_tensor("drel", [128, NCOL], bf16, kind="ExternalInput")
    out = nc.dram_tensor("out", [NLOC, 40], f32, kind="ExternalOutput")

    agi1 = nc.dram_tensor("agi1", [NLOC, 128], f16)    # local [h1|as1|ad1|pad]
    tab1 = nc.dram_tensor("tab1", [NTOT, 128], f16, addr_space="Shared")
    agi2 = nc.dram_tensor("agi2", [NLOC, 128], f16)    # [h2|as2|ad2|pad]
    ago = nc.dram_tensor("ago", [NTOT, 128], f16, addr_space="Shared")

    def BC(ap, dims):
        return bass.AP(ap.tensor, ap.offset, dims)

    def dram_rows(t, offset_rows, dims):
        """AP into DRAM tensor t (row-major, 128 f16 cols) at row offset."""
        return bass.AP(t, offset_rows * 128, dims)

    with tile.TileContext(nc) as tc:
        with tc.tile_pool(name="const", bufs=1) as pc:
            nc.gpsimd.load_library(mlp)

            drel_sb = pc.tile([128, NCOL], bf16)
            nc.sync.dma_start(drel_sb[:], drel_d[:])
            w1_sb = pc.tile([128, 2, 80], f16)
            nc.sync.dma_start(w1_sb[:], W1e[:].rearrange("k p n -> p k n"))
            w2_sb = pc.tile([64, 42], f16)
            nc.sync.dma_start(w2_sb[:], W2e[:])

            NIOTA = max(MAXCPW, ATSEGC)
            ii = pc.tile([128, 128, NIOTA], i16)
            nc.gpsimd.iota(ii[:], pattern=[[1, 128], [0, NIOTA]], base=0,
                           channel_multiplier=0)
            iota_rep = pc.tile([128, 128, MAXCPW], bf16)
            nc.vector.tensor_copy(out=iota_rep[:], in_=ii[:, :, 0:MAXCPW])
            iota_at = pc.tile([128, 128, ATSEGC], f16)
            nc.vector.tensor_copy(out=iota_at[:], in_=ii[:, :, 0:ATSEGC])
            startd_sb = pc.tile([128, NCOLAT], f16)
            nc.sync.dma_start(startd_sb[:], startd_d[:])
            mshift_sb = pc.tile([128, 2, 128], f16)
            nc.sync.dma_start(mshift_sb[:], mshift_d[:])

            ident = pc.tile([128, 128], f16)
            make_identity(nc, ident[:])

            ones32 = pc.tile([1, 128], f32)
            nc.vector.memset(ones32[:], 1.0)

            b1r = pc.tile([128, 64], f32)
            b2r = pc.tile([128, 40], f32)
            with tc.tile_pool(name="pini", bufs=2, space="PSUM") as ppi:
                for row_d, width, dest in ((b1, 64, b1r), (b2, 40, b2r)):
                    t = pc.tile([1, width], f32, tag=f"rrow{width}")
                    nc.sync.dma_start(t[:], row_d[:])
                    ps = ppi.tile([128, width], f32, tag="rep")
                    nc.tensor.matmul(ps[:], lhsT=ones32[:], rhs=t[:],
                                     start=True, stop=True)
                    nc.vector.tensor_copy(out=dest[:], in_=ps[:])

            # a_dst per-slot tiles, telescoped from the own-block a_dst
            # values via PE (P[d,s] = (s >= start[d]) is a step matrix;
            # P @ diff(ad) = ad[drel[s]] since slots are drel-sorted)
            at1_all = pc.tile([128, plan["NCHUNK"], 8], f16)
            at2_all = pc.tile([128, plan["NCHUNK"], 1], f16)
            ad2own = pc.tile([128, NW + 1, 1], f16)
            nc.vector.memset(ad2own[:, 0:1, :].rearrange("p a b -> p (a b)"),
                             0.0)

            def at_fill(dall, at_tile, H, pat, ppat):
                for (kg0, nk, c0, ncols, cols) in atsegs:
                    P = pat.tile([128, 128, ATSEGC], f16, tag="P")
                    st_ = startd_sb[:, c0:c0 + ncols]
                    nc.vector.tensor_tensor(
                        out=P[:, :, 0:ncols],
                        in0=iota_at[:, :, 0:ncols],
                        in1=BC(st_, [st_.ap[0], [0, 128], st_.ap[1]]),
                        op=OP.is_ge)
                    ps = ppat.tile([128, ATSEGC, 8], f32, tag="atps")
                    for (i, klocal, v, w, first, last) in cols:
                        nc.tensor.matmul(ps[:, klocal, 0:H],
                                         lhsT=P[:, :, i],
                                         rhs=dall[:, v, w, 0:H],
                                         start=first, stop=last)
                    nc.scalar.activation(out=at_tile[:, kg0:kg0 + nk, :],
                                         in_=ps[:, 0:nk, 0:H], func=AF.Copy)

            def build_dall(adown, H, dall, ppd):
                # adown: [128, NW+1, H] f16, col 0 zeroed.
                # dall[:,0,w,:] = fwd-diff (row d: ad[d]-ad[d-1], row0 ad[0]),
                # dall[:,1,w,:] = same but row0 = ad[0,w]-ad[127,w-1].
                # PSUM rows padded to whole banks (512 f32); each matmul's
                # output chunk must stay inside one bank
                psrow = ((NW * H + 511) // 512) * 512
                ps = ppd.tile([128, 2, psrow], f32, tag=f"dps{H}")
                a_ = adown[:]
                for off in range(0, NW * H, 512):
                    nn = min(512, NW * H - off)
                    rhs_cur = bass.AP(a_.tensor, a_.offset + H + off,
                                      [a_.ap[0], [1, nn]])
                    rhs_prev = bass.AP(a_.tensor, a_.offset + off,
                                       [a_.ap[0], [1, nn]])
                    nc.tensor.matmul(ps[:, 0, off:off + nn],
                                     lhsT=mshift_sb[:, 0, :],
                                     rhs=rhs_cur, start=True, stop=True)
                    nc.tensor.matmul(ps[:, 1, off:off + nn],
                                     lhsT=mshift_sb[:, 0, :],
                                     rhs=rhs_cur, start=True, stop=False)
                    nc.tensor.matmul(ps[:, 1, off:off + nn],
                                     lhsT=mshift_sb[:, 1, :],
                                     rhs=rhs_prev, start=False, stop=True)
                nc.scalar.activation(
                    out=dall[:].rearrange("p a b c -> p a (b c)"),
                    in_=ps[:, :, 0:NW * H], func=AF.Copy)

            # ---------- phase A (sharded): each core transforms only its own
            # node block -> agi1, AllGather -> tab1; the at1 telescoping runs
            # during the collective (it reads only the local agi1) ----------
            with (tc.tile_pool(name="pa", bufs=3) as pa,
                  tc.tile_pool(name="ppa", bufs=2, space="PSUM") as ppa):
                AB = 10                      # chunks per DMA batch
                for jj in range(0, 100, AB):
                    nb = min(AB, 100 - jj)
                    xt = pa.tile([128, 2, AB * 128], f16, tag="xt")
                    for k in range(2):
                        nc.sync.dma_start(
                            xt[:, k, 0:nb * 128],
                            xTloc[k, :, jj * 128:(jj + nb) * 128])
                    row = pa.tile([128, AB, 128], f16, tag="row")
                    for u in range(0, nb, 4):
                        ub = min(4, nb - u)
                        ps = ppa.tile([128, 4, 80], f32, tag="np1")
                        for j in range(ub):
                            for k in range(2):
                                nc.tensor.matmul(
                                    ps[:, j, :],
                                    lhsT=xt[:, k,
                                            (u + j) * 128:(u + j + 1) * 128],
                                    rhs=w1_sb[:, k, :], start=(k == 0),
                                    stop=(k == 1))
                        if (u // 4) % 2:
                            nc.vector.tensor_copy(out=row[:, u:u + ub, 0:80],
                                                  in_=ps[:, 0:ub, :])
                        else:
                            nc.scalar.copy(out=row[:, u:u + ub, 0:80],
                                           in_=ps[:, 0:ub, :])
                    nc.sync.dma_start(
                        dram_rows(agi1, jj,
                                  [[NW * 128, 128], [256, nb // 2],
                                   [1, 256]]),
                        row[:, 0:nb, :])

                nc.gpsimd.collective_compute(
                    "AllGather", OP.bypass, ins=[agi1[:]], outs=[tab1[:]],
                    replica_groups=[list(range(8))])

            # at1 telescoping (reads local agi1; overlaps the AllGather)
            with (tc.tile_pool(name="pat1", bufs=2) as pat1,
                  tc.tile_pool(name="ppat1", bufs=2, space="PSUM") as ppat1,
                  tc.tile_pool(name="ppd1", bufs=1, space="PSUM") as ppd1):
                ad1own = pat1.tile([128, NW + 1, 8], f16, tag="adown")
                nc.vector.memset(
                    ad1own[:, 0:1, :].rearrange("p a b -> p (a b)"), 0.0)
                nc.sync.dma_start(
                    ad1own[:, 1:NW + 1, :],
                    bass.AP(agi1, 72, [[NW * 128, 128], [128, NW], [1, 8]]))
                dall1 = pat1.tile([128, 2, NW, 8], f16, tag="dall")
                build_dall(ad1own, 8, dall1, ppd1)
                at_fill(dall1, at1_all, 8, pat1, ppat1)

            # ---------- phase B: layer-1 edge pass ----------
            def edge_pass(layer):
                if layer == 1:
                    tab, ncols_h, as_col = tab1, 72, 64
                    nheads, msgw = 8, 72
                else:
                    tab, ncols_h, as_col = ago, 41, 40
                    nheads, msgw = 1, 41
                pool_name = f"pe{layer}"
                with (tc.tile_pool(name=pool_name, bufs=2) as pb,
                      tc.tile_pool(name=pool_name + "h", bufs=2) as ph,
                      tc.tile_pool(name=pool_name + "m", bufs=1) as pm,
                      tc.tile_pool(name=pool_name + "w",
                                   bufs=(2 if layer == 1 else 3)) as pw,
                      tc.tile_pool(name=pool_name + "p", bufs=2,
                                   space="PSUM") as ppb):
                    qn = 0
                    for s in range(NSB):
                        k0, k1 = sb_chunks[s]
                        cps = k1 - k0
                        hix = pb.tile([128, CPSB_MAX * 8], i16, tag="hix")
                        nc.sync.dma_start(hix[:, 0:cps * 8],
                                          hidx_d[:, k0 * 8:k1 * 8])

                        ht = ph.tile([128, CPSB_MAX, ncols_h], f16, tag="ht")
                        for (ss, q, slot0, nids) in plan["hcalls"]:
                            if ss != s:
                                continue
                            c0 = slot0 // 128 - k0
                            _gather_small(
                                nc.gpsimd,
                                ht[:, c0:c0 + nids // 128, :],
                                tab[q * QS:(q + 1) * QS, 0:ncols_h],
                                hix[:, (slot0 - k0 * 128) // 16:
                                    (slot0 - k0 * 128 + nids) // 16],
                                nids, ncols_h, 128, queue_num=qn % 4)
                            qn += 1
                        if layer == 1:
                            at_s = at1_all[:, k0:k1, :]
                        else:
                            at_s = at2_all[:, k0:k1, 0:1]

                        e = pm.tile([128, CPSB_MAX, nheads], f32, tag="e")
                        lr = e
                        nc.vector.tensor_tensor(
                            out=e[:, 0:cps, :],
                            in0=ht[:, 0:cps, as_col:as_col + nheads],
                            in1=at_s, op=OP.add)
                        nc.vector.scalar_tensor_tensor(
                            out=lr[:, 0:cps, :], in0=e[:, 0:cps, :],
                            scalar=0.2, in1=e[:, 0:cps, :],
                            op0=OP.mult, op1=OP.max)

                        msg = pm.tile([128, CPSB_MAX, msgw], bf16, tag="msg")
                        # w into msg's trailing cols (compact exp)
                        nc.scalar.activation(
                            out=msg[:, 0:cps, as_col:as_col + nheads],
                            in_=lr[:, 0:cps, :], func=AF.Exp)
                        if layer == 1:
                            # expanded weights for a clean 2x-mode mult
                            half = (CPSB_MAX + 1) // 2
                            wgx = pm.tile([128, half, 8, 8], bf16, tag="wgx")
                            for h0 in (0, half):
                                hn = min(half, cps - h0)
                                if hn <= 0:
                                    continue
                                lrs = lr[:, h0:h0 + hn, :]
                                nc.scalar.activation(
                                    out=wgx[:, 0:hn, :, :],
                                    in_=BC(lrs, [lrs.ap[0], lrs.ap[1],
                                                 lrs.ap[2], [0, 8]]),
                                    func=AF.Exp)
                                m_ = msg[:, h0:h0 + hn, 0:64]
                                h_ = ht[:, h0:h0 + hn, 0:64]
                                nc.vector.tensor_tensor(
                                    out=BC(m_, [m_.ap[0], m_.ap[1],
                                                [8, 8], [1, 8]]),
                                    in0=BC(h_, [h_.ap[0], h_.ap[1],
                                                [8, 8], [1, 8]]),
                                    in1=wgx[:, 0:hn, :, :], op=OP.mult)
                        else:
                            wgx2 = pw.tile([128, CPSB_MAX, 40], bf16,
                                           tag="wgx2")
                            lrs = lr[:, 0:cps, :]
                            nc.scalar.activation(
                                out=wgx2[:, 0:cps, :],
                                in_=BC(lrs, [lrs.ap[0], lrs.ap[1], [0, 40]]),
                                func=AF.Exp)
                            nc.vector.tensor_tensor(
                                out=msg[:, 0:cps, 0:40],
                                in0=ht[:, 0:cps, 0:40],
                                in1=wgx2[:, 0:cps, :], op=OP.mult)

                        # windows: one-hot + aggregation matmuls, PSUM
                        # evicted into a per-sb batch tile
                        hsb = pm.tile([128, NWSB, msgw], f32, tag="hsb")
                        for wi in range(NWSB):
                            w = s * NWSB + wi
                            cols = wcols[w]
                            cpw = len(cols)
                            c0 = colbase[w]
                            ohT = pw.tile([128, 128, MAXCPW], bf16, tag="ohT")
                            dr = drel_sb[:, c0:c0 + cpw]
                            nc.vector.tensor_tensor(
                                out=ohT[:, :, 0:cpw],
                                in0=BC(dr, [dr.ap[0], [0, 128], dr.ap[1]]),
                                in1=iota_rep[:, :, 0:cpw], op=OP.is_equal)
                            ps = ppb.tile([128, msgw], f32, tag="agg")
                            for i, k in enumerate(cols):
                                nc.tensor.matmul(
                                    ps[:], lhsT=ohT[:, :, i],
                                    rhs=msg[:, k - k0, :],
                                    start=(i == 0), stop=(i == cpw - 1))
                            nc.scalar.copy(out=hsb[:, wi, :], in_=ps[:])

                        # per-sb batched softmax-normalize (+ elu/r2 for L1)
                        if layer == 1:
                            den = pw.tile([128, NWSB, 8], f32, tag="den")
                            nc.scalar.activation(out=den[:],
                                                 in_=hsb[:, :, 64:72],
                                                 func=AF.Copy, bias=ACC_EPS)
                            rec = pw.tile([128, NWSB, 8], f32, tag="rec")
                            nc.vector.reciprocal(
                                rec[:].rearrange("p a b -> p (a b)"),
                                den[:].rearrange("p a b -> p (a b)"))
                            o1 = pw.tile([128, NWSB, 64], f32, tag="o1")
                            nu = hsb[:, :, 0:64]
                            r_ = rec[:]
                            nc.vector.tensor_tensor(
                                out=BC(o1[:], [o1[:].ap[0], [64, NWSB],
                                               [8, 8], [1, 8]]),
                                in0=BC(nu, [nu.ap[0], [72, NWSB],
                                            [8, 8], [1, 8]]),
                                in1=BC(r_, [r_.ap[0], [8, NWSB],
                                            [1, 8], [0, 8]]),
                                op=OP.mult)
                            o1v = o1[:].rearrange("p a b -> p (a b)")
                            if not plan["skip_b1"]:
                                b1w = b1r[:]
                                nc.vector.tensor_tensor(
                                    out=o1v,
                                    in0=o1v,
                                    in1=BC(b1w, [b1w.ap[0], [0, NWSB],
                                                 [1, 64]]),
                                    op=OP.add)
                            # elu = relu(x) + exp(-relu(-x)) - 1
                            rneg = pw.tile([128, NWSB, 64], f32, tag="rneg")
                            nc.scalar.activation(
                                out=rneg[:].rearrange("p a b -> p (a b)"),
                                in_=o1v, func=AF.Relu, scale=-1.0)
                            expn = rneg
                            nc.scalar.activation(
                                out=expn[:].rearrange("p a b -> p (a b)"),
                                in_=rneg[:].rearrange("p a b -> p (a b)"),
                                func=AF.Exp, scale=-1.0)
                            pos = pw.tile([128, NWSB, 64], f32, tag="pos")
                            nc.scalar.activation(
                                out=pos[:].rearrange("p a b -> p (a b)"),
                                in_=o1v, func=AF.Relu)
                            hl16 = pw.tile([128, NWSB, 64], f16, tag="hl16")
                            nc.vector.scalar_tensor_tensor(
                                out=hl16[:].rearrange("p a b -> p (a b)"),
                                in0=expn[:].rearrange("p a b -> p (a b)"),
                                scalar=-1.0,
                                in1=pos[:].rearrange("p a b -> p (a b)"),
                                op0=OP.add, op1=OP.add)
                            r2s = pw.tile([128, NWSB, 42], f16, tag="r2s")
                            for wi in range(NWSB):
                                pst = ppb.tile([64, 128], f16, tag="tr")
                                nc.tensor.transpose(out=pst[:],
                                                    in_=hl16[:, wi, :],
                                                    identity=ident[:])
                                hlT = pw.tile([64, 128], f16, tag="hlT")
                                nc.scalar.copy(out=hlT[:], in_=pst[:])
                                r2p = ppb.tile([128, 42], f32, tag="r2p")
                                nc.tensor.matmul(r2p[:], lhsT=hlT[:],
                                                 rhs=w2_sb[:], start=True,
                                                 stop=True)
                                nc.scalar.copy(out=r2s[:, wi, :], in_=r2p[:])
                            nc.sync.dma_start(
                                bass.AP(agi2, (s * NWSB) * 128,
                                        [[NW * 128, 128], [128, NWSB],
                                         [1, 42]]),
                                r2s[:])
                            # stash a_dst2 (col 41) for the L2 telescoping
                            nc.scalar.copy(
                                out=ad2own[:, 1 + s * NWSB:
                                           1 + (s + 1) * NWSB, :],
                                in_=r2s[:, :, 41:42])
                        else:
                            den = pw.tile([128, NWSB, 1], f32, tag="den2")
                            nc.scalar.activation(out=den[:],
                                                 in_=hsb[:, :, 40:41],
                                                 func=AF.Copy, bias=ACC_EPS)
                            rec = pw.tile([128, NWSB, 1], f32, tag="rec2")
                            nc.vector.reciprocal(
                                rec[:].rearrange("p a b -> p (a b)"),
                                den[:].rearrange("p a b -> p (a b)"))
                            o2 = pw.tile([128, NWSB, 40], f32, tag="o2")
                            nu = hsb[:, :, 0:40]
                            r_ = rec[:]
                            nc.vector.tensor_tensor(
                                out=o2[:],
                                in0=BC(nu, [nu.ap[0], [41, NWSB], [1, 40]]),
                                in1=BC(r_, [r_.ap[0], [1, NWSB], [0, 40]]),
                                op=OP.mult)
                            o2v = o2[:].rearrange("p a b -> p (a b)")
                            if not plan["skip_b2"]:
                                b2w = b2r[:]
                                nc.vector.tensor_tensor(
                                    out=o2v, in0=o2v,
                                    in1=BC(b2w, [b2w.ap[0], [0, NWSB],
                                                 [1, 40]]),
                                    op=OP.add)
                            nc.sync.dma_start(
                                bass.AP(out, (s * NWSB) * 128 * 40,
                                        [[40, 128], [128 * 40, NWSB],
                                         [1, 40]]),
                                o2[:])

            edge_pass(1)

            # ---------- AllGather first; the at2 telescoping (local data
            # only) runs during the collective ----------
            nc.gpsimd.collective_compute(
                "AllGather", OP.bypass, ins=[agi2[:]], outs=[ago[:]],
                replica_groups=[list(range(8))])

            with (tc.tile_pool(name="pat2", bufs=2) as pat2,
                  tc.tile_pool(name="ppat2", bufs=2, space="PSUM") as ppat2,
                  tc.tile_pool(name="ppd2", bufs=1, space="PSUM") as ppd2):
                dall2 = pat2.tile([128, 2, NW, 1], f16, tag="dall")
                build_dall(ad2own, 1, dall2, ppd2)
                at_fill(dall2, at2_all, 1, pat2, ppat2)

            edge_pass(2)

    nc.finalize()
    return nc


def kernel(**inputs):
    per_core, plan = _host_prep(**inputs)
    if "nc" not in _CACHE:
        _CACHE["nc"] = _build_nc(plan)
    nc = _CACHE["nc"]
    from concourse.bass_utils import run_bass_kernel_spmd
    res = run_bass_kernel_spmd(nc, per_core, list(range(8)))
    full = np.concatenate([res.results[c]["out"] for c in range(8)], axis=0)
    return np.ascontiguousarray(full[:N]).astype(np.float32)



# revision 59
# speedup vs baseline: 1.0388x; 1.0388x over previous
"""GAT (2-layer PyG GATConv, eval) on 8 Trainium2 NeuronCores.

Sharding: nodes range-partitioned (NLOC=12800/core); core c owns edges whose
dst is in its range. The layer-1 node table is REPLICATED: every core
transforms all 102400 rows from the full x (the x stream + table write cost
less on the DMA timeline than the AllGather they replace, and the a_dst
telescoping runs concurrently on DVE from the core's own x shard). Layer 2's
table still needs one AllGather (agi2 -> ago), overlapped by the at2
telescoping machinery.

Slot layout per core: superblock (10 windows) -> quadrant -> window, with
per-(window,quadrant) STATIC capacities = max edge count over the 8 cores
(SPMD: one module runs on all cores; only tensor contents differ) — ~10%
slot padding vs 28% for fixed-size groups. Gather calls use
single_packet=False, which lets the SWDGE ucode stream descriptors through
the ring: up to 4x the ring (dynamic_dma_scratch_size/16) indices per call
(probed on HW: 8192 idx streams through a 2048-desc ring; 8x crashes;
single-packet calls hard-crash above 1024 idx). One ~5.5k-idx call per
(sb,quadrant) segment cuts the 994ns-per-call Pool overhead ~5x vs the
1024-idx baseline.

Per layer, per edge slot: a 144B/82B payload gather pulls [h|a_src] rows
(256B-stride tables, int16 idx into 25600-row quadrants). a_dst is NOT
gathered: slots are drel-sorted inside each (window,quadrant) group, so
per-slot a_dst[drel] telescopes — P[d,s] = (s >= start[d]) is a DVE is_ge
step matrix (host-static start tables) and PE computes P^T @ diff(a_dst)
per chunk column; window starts mid-chunk use a compensated diff table
(row0 = ad[0,w]-ad[127,w-1]) accumulated in the same PSUM. Both at1 and at2
expansions run during their layer's AllGather (they read only local data).
Softmax is the shift-invariant no-max form (w = exp(leakyrelu(as+ad)),
|e| < ~25 so fp32 exp is safe). Segment reduction is a PE matmul whose
stationary matrix is a transposed one-hot built by DVE is_equal in the
2x-mode layout; boundary chunks carry a masked drel column per touching
window. msg = h*w uses an Act-expanded weight tile so the DVE mult runs in
2x mode. Softmax normalize + ELU + the r2 = hlT @ [W2|a2_src|a2_dst]
projection are batched per superblock; tables are written node-permuted
(row = p*100 + j within each core block) so writes coalesce per partition.
"""
import numpy as np
import ml_dtypes

N = 100000
E = 1600000
NF = 256
HEADS, NHID = 8, 8
NH = HEADS * NHID          # 64
NCLASS = 40
NLOC = 12800               # nodes per core
NW = 100                   # 128-dst windows per core
NQ = 4                     # src table quadrants
QS = 25600                 # rows per quadrant
NWSB = 10                  # windows per superblock
NSB = NW // NWSB           # 10 superblocks
NTOT = 102400
ACC_EPS = 1e-16

_CACHE = {}


def _ceil128(x):
    return (x + 127) & ~127


def _host_prep(x, edge_index, W1, a1_src, a1_dst, b1, W2, a2_src, a2_dst, b2):
    src = np.asarray(edge_index[0], dtype=np.int64)
    dst = np.asarray(edge_index[1], dtype=np.int64)

    # table-row permutation: node n -> row  c*NLOC + (l%128)*NW + l//128
    def rowperm(n):
        c = n // NLOC
        l = n - c * NLOC
        return c * NLOC + (l % 128) * NW + l // 128

    srow = rowperm(src)
    sq = srow // QS
    sidx = (srow - sq * QS).astype(np.int16)

    core = dst // NLOC
    dloc = dst - core * NLOC
    w_e = (dloc >> 7).astype(np.int64)
    dr_e = (dloc & 127).astype(np.int64)
    adidx = (dr_e * NW + w_e).astype(np.int16)
    sb_e = w_e // NWSB

    # static capacities: max over cores per (window, quadrant); >=1 so every
    # group is present in the slot stream (the telescoped a_dst expansion
    # needs window w-1 to precede window w inside each (sb,q) segment)
    gkey = (core * NW + w_e) * NQ + sq          # [E]
    cnt = np.bincount(gkey, minlength=8 * NW * NQ).reshape(8, NW, NQ)
    cap = np.maximum(cnt.max(axis=0), 1)         # [NW, NQ]

    # slot layout: sb -> quadrant -> window.  With single_packet=False the
    # SWDGE gather ucode streams descriptors through the ring, so calls up
    # to 2x the ring size (dynamic_dma_scratch_size/16) are fine (probed on
    # HW: 8192 idx with a 4096-desc ring).
    MAXIDX = 8192
    wq_start = np.zeros((NW, NQ), np.int64)
    hcalls = []                                  # (sb, q, slot0, n_idx)
    sb_chunks = []                               # (k0, k1) per sb
    nslot = 0
    for s in range(NSB):
        k0 = nslot // 128
        for q in range(NQ):
            seg0 = nslot
            for w in range(s * NWSB, (s + 1) * NWSB):
                wq_start[w, q] = nslot
                nslot += int(cap[w, q])
            nslot = _ceil128(nslot)
            for off in range(seg0, nslot, MAXIDX):
                hcalls.append((s, q, off, min(MAXIDX, nslot - off)))
        sb_chunks.append((k0, nslot // 128))
    NSLOT = nslot
    NCHUNK = NSLOT // 128

    # window label per slot (shared across cores: layout is static).  Group
    # pads inherit their group's window; (sb,q)-tail ceil128 pads inherit the
    # segment's last window.
    wfull = np.zeros(NSLOT, np.int64)
    for s in range(NSB):
        for q in range(NQ):
            for w in range(s * NWSB, (s + 1) * NWSB):
                a = int(wq_start[w, q])
                e_ = a + int(cap[w, q])
                wfull[a:e_] = w
            wfull[e_:_ceil128(e_)] = (s + 1) * NWSB - 1   # segment tail pads

    # at-plan: per chunk, one column-copy per touching window; copies of one
    # chunk accumulate in PSUM.  v=1 (compensated diff table) iff the window
    # starts mid-chunk.  Packed into segments of <= ATSEGC columns.
    ATSEGC = 24
    atsegs = []            # (kg0, nk, c0, cols=[(i, klocal, v, w, first, last)])
    copies_per_chunk = []  # [(w, a, b)]
    for k in range(NCHUNK):
        wk = wfull[k * 128:(k + 1) * 128]
        runs = []
        a = 0
        for p in range(1, 128):
            if wk[p] != wk[p - 1]:
                runs.append((int(wk[a]), a, p))
                a = p
        runs.append((int(wk[a]), a, 128))
        copies_per_chunk.append(runs)
    ncolat = 0
    cur = None
    colat_of = {}
    for k in range(NCHUNK):
        runs = copies_per_chunk[k]
        if cur is None or cur[3] + len(runs) > ATSEGC:
            if cur is not None:
                atsegs.append(cur)
            cur = [k, 0, ncolat, 0, []]
        kloc = cur[1]
        for i, (w, a, b) in enumerate(runs):
            colat_of[(k, a)] = ncolat
            cur[4].append((cur[3], kloc, 1 if a > 0 else 0, w,
                           i == 0, i == len(runs) - 1))
            cur[3] += 1
            ncolat += 1
        cur[1] += 1
    if cur is not None:
        atsegs.append(cur)
    NCOLAT = ncolat

    # per-window chunk columns
    colmap = np.full((NW, NCHUNK), -1, np.int64)
    wcols = []                                   # per w: (colbase, [chunks])
    ncol = 0
    for w in range(NW):
        cols = []
        for q in range(NQ):
            a = int(wq_start[w, q])
            b = a + int(cap[w, q])
            for k in range(a // 128, (b + 127) // 128):
                cols.append(k)
                colmap[w, k] = ncol
                ncol += 1
        wcols.append(cols)
    NCOL = ncol
    MAXCPW = max(len(c) for c in wcols)

    plan = {
        "NSLOT": NSLOT, "NCHUNK": NCHUNK, "NCOL": NCOL, "MAXCPW": MAXCPW,
        "NCOLAT": NCOLAT, "ATSEGC": ATSEGC,
        "hcalls": hcalls, "atsegs": atsegs, "sb_chunks": sb_chunks,
        "wcols": wcols,
        "skip_b1": bool(np.all(np.asarray(b1) == 0)),
        "skip_b2": bool(np.all(np.asarray(b2) == 0)),
    }

    # group-id in slot order: (sb, q, w_in_sb)
    flatg = (sb_e * NQ + sq) * NWSB + (w_e - sb_e * NWSB)
    gstart_flat = np.zeros(NSB * NQ * NWSB, np.int64)
    for s in range(NSB):
        for q in range(NQ):
            for wi in range(NWSB):
                gstart_flat[(s * NQ + q) * NWSB + wi] = wq_start[s * NWSB + wi, q]

    per_core = []
    hidx_all, startd_all, drel_all = [], [], []
    for c in range(8):
        m = core == c
        fg = flatg[m]
        drc = dr_e[m]
        order = np.lexsort((drc, fg))
        fgs = fg[order]
        cntc = np.bincount(fgs, minlength=NSB * NQ * NWSB)
        starts = np.zeros_like(cntc)
        starts[1:] = np.cumsum(cntc)[:-1]
        rank = np.arange(len(fgs)) - starts[fgs]
        slot = gstart_flat[fgs] + rank

        hvec = np.zeros(NSLOT, np.int16)
        hvec[slot] = sidx[m][order]

        drel = np.full((128, NCOL), 128.0, np.float32)
        k_s = slot >> 7
        p_s = slot & 127
        we_s = w_e[m][order]
        col_s = colmap[we_s, k_s]
        assert (col_s >= 0).all()
        drel[p_s, col_s] = drc[order].astype(np.float32)

        # per-slot drel stream (pads = 128) for the telescoped start tables
        drfull = np.full(NSLOT, 128, np.int64)
        drfull[slot] = drc[order]
        startd = np.zeros((128, NCOLAT), np.float32)
        dgrid = np.arange(128)
        for k in range(NCHUNK):
            for (w, a, b) in copies_per_chunk[k]:
                col = colat_of[(k, a)]
                drs = drfull[k * 128 + a:k * 128 + b]
                startd[:, col] = a + np.searchsorted(drs, dgrid)

        def wrap16(v):
            o = np.zeros((128, NSLOT // 16), np.int16)
            sl = np.arange(NSLOT)
            o[sl % 16, sl // 16] = v
            for r in range(1, 8):
                o[16 * r:16 * (r + 1)] = o[:16]
            return o

        hidx_all.append(wrap16(hvec))
        startd_all.append(startd.astype(np.float16))
        drel_all.append(drel.astype(ml_dtypes.bfloat16))

    # weights
    W1 = np.asarray(W1, np.float32)
    v_s1 = np.einsum("chk,hk->ch", W1.reshape(NF, HEADS, NHID),
                     np.asarray(a1_src, np.float32))
    v_d1 = np.einsum("chk,hk->ch", W1.reshape(NF, HEADS, NHID),
                     np.asarray(a1_dst, np.float32))
    W1e = np.concatenate([W1, v_s1, v_d1], axis=1).reshape(2, 128, 80)
    W1e = W1e.astype(np.float16)

    W2 = np.asarray(W2, np.float32)
    v_s2 = W2 @ np.asarray(a2_src, np.float32)[0]
    v_d2 = W2 @ np.asarray(a2_dst, np.float32)[0]
    W2e = np.concatenate([W2, v_s2[:, None], v_d2[:, None]],
                         axis=1).astype(np.float16)   # [64, 42]

    xp = np.zeros((NTOT, NF), np.float32)
    xp[:N] = np.asarray(x, np.float32)

    # lhsT matrices for building the diff tables on PE:
    # mshift[:,0,:] = Mplain^T (fwd diff), mshift[:,1,:] = -sel(127)->row0
    mshiftT = np.zeros((128, 2, 128), np.float16)
    mshiftT[:, 0, :] = (np.eye(128) - np.eye(128, k=1)).astype(np.float16)
    mshiftT[127, 1, 0] = -1.0

    # full transposed x, shared by all cores: the layer-1 table transform is
    # replicated (each core computes all 102400 rows locally; the x stream +
    # table write fit under what the AllGather used to cost, and the a_dst
    # telescoping runs concurrently from the core's own x shard)
    xT = np.ascontiguousarray(xp.T).astype(np.float16).reshape(2, 128, NTOT)

    for c in range(8):
        xloc = np.ascontiguousarray(xp[c * NLOC:(c + 1) * NLOC].T)
        per_core.append({
            "xTloc": xloc.astype(np.float16).reshape(2, 128, NLOC),
            "xTfull": xT,
            "W1e": W1e,
            "W2e": W2e,
            "b1": np.asarray(b1, np.float32)[None, :],
            "b2": np.asarray(b2, np.float32)[None, :],
            "hidx": hidx_all[c],
            "startd": startd_all[c],
            "drel": drel_all[c],
            "mshift": mshiftT,
        })
    return per_core, plan


def _gather_small(g, out_ap, in_ap, idxs_ap, num_idxs, elem_size, elem_step,
                  queue_num=0, single_packet=False):
    """dma_gather with payload < 256B; only the 256B row-stride rule is real
    for the non-transpose path."""
    import concourse.mybir as mybir
    stride_bytes = elem_step * mybir.dt.size(in_ap.dtype)
    assert stride_bytes % 256 == 0
    _in_ap = g.lower_ap_dma(in_ap, for_custom_bir_dma=True)
    _idxs_ap = g.lower_ap(idxs_ap)
    _out_ap = g.lower_ap(out_ap)
    return g.add_instruction(mybir.InstDMAGatherAnt(
        name=g.bass.get_next_instruction_name(),
        ins=[*_in_ap, _idxs_ap, g.lower_val_access(g.to_reg(num_idxs))],
        outs=[_out_ap],
        transpose=False,
        num_idxs=num_idxs,
        elem_size=elem_size,
        stride_bytes_256=stride_bytes // 256,
        gen_mode=0,
        single_packet=single_packet,
        queue_num=queue_num,
        sbuf_tokens_per_rank=0,
        sbuf_free_dim_per_rank=0,
        sbuf_free_dim_pad_per_rank=0,
        sbuf_byte_offset=0,
    ))


def _build_nc(plan):
    import concourse.bass as bass
    import concourse.bacc as bacc
    import concourse.mybir as mybir
    import concourse.tile as tile
    from concourse.library_config import mlp
    from concourse.masks import make_identity

    f32, f16, bf16, i16 = (mybir.dt.float32, mybir.dt.float16,
                           mybir.dt.bfloat16, mybir.dt.int16)
    AF = mybir.ActivationFunctionType
    OP = mybir.AluOpType

    NSLOT = plan["NSLOT"]
    NCOL = plan["NCOL"]
    MAXCPW = plan["MAXCPW"]
    sb_chunks = plan["sb_chunks"]
    wcols = plan["wcols"]
    CPSB_MAX = max(k1 - k0 for k0, k1 in sb_chunks)
    colbase = [0] * NW
    for w in range(1, NW):
        colbase[w] = colbase[w - 1] + len(wcols[w - 1])

    NCOLAT = plan["NCOLAT"]
    ATSEGC = plan["ATSEGC"]
    atsegs = plan["atsegs"]

    nc = bacc.Bacc("TRN2", target_bir_lowering=False, debug=False,
                   num_devices=8, num_swdge_queues=4,
                   dynamic_dma_scratch_size=32768)

    xTloc = nc.dram_tensor("xTloc", [2, 128, NLOC], f16, kind="ExternalInput")
    xTfull = nc.dram_tensor("xTfull", [2, 128, NTOT], f16,
                            kind="ExternalInput")
    W1e = nc.dram_tensor("W1e", [2, 128, 80], f16, kind="ExternalInput")
    W2e = nc.dram_tensor("W2e", [64, 42], f16, kind="ExternalInput")
    b1 = nc.dram_tensor("b1", [1, 64], f32, kind="ExternalInput")
    b2 = nc.dram_tensor("b2", [1, 40], f32, kind="ExternalInput")
    hidx_d = nc.dram_tensor("hidx", [128, NSLOT // 16], i16,
                            kind="ExternalInput")
    startd_d = nc.dram_tensor("startd", [128, NCOLAT], f16,
                              kind="ExternalInput")
    mshift_d = nc.dram_tensor("mshift", [128, 2, 128], f16,
                              kind="ExternalInput")
    drel_d = nc.dram_tensor("drel", [128, NCOL], bf16, kind="ExternalInput")
    out = nc.dram_tensor("out", [NLOC, 40], f32, kind="ExternalOutput")

    tab1 = nc.dram_tensor("tab1", [NTOT, 128], f16)    # [h1|as1|ad1|pad] rows
    agi2 = nc.dram_tensor("agi2", [NLOC, 128], f16)    # [h2|as2|ad2|pad]
    ago = nc.dram_tensor("ago", [NTOT, 128], f16, addr_space="Shared")

    def BC(ap, dims):
        return bass.AP(ap.tensor, ap.offset, dims)

    def dram_rows(t, offset_rows, dims):
        """AP into DRAM tensor t (row-major, 128 f16 cols) at row offset."""
        return bass.AP(t, offset_rows * 128, dims)

    with tile.TileContext(nc) as tc:
        with tc.tile_pool(name="const", bufs=1) as pc:
            nc.gpsimd.load_library(mlp)

            drel_sb = pc.tile([128, NCOL], bf16)
            nc.sync.dma_start(drel_sb[:], drel_d[:])
            w1_sb = pc.tile([128, 2, 80], f16)
            nc.sync.dma_start(w1_sb[:], W1e[:].rearrange("k p n -> p k n"))
            w2_sb = pc.tile([64, 42], f16)
            nc.sync.dma_start(w2_sb[:], W2e[:])

            NIOTA = max(MAXCPW, ATSEGC)
            ii = pc.tile([128, 128, NIOTA], i16)
            nc.gpsimd.iota(ii[:], pattern=[[1, 128], [0, NIOTA]], base=0,
                           channel_multiplier=0)
            iota_rep = pc.tile([128, 128, MAXCPW], bf16)
            nc.vector.tensor_copy(out=iota_rep[:], in_=ii[:, :, 0:MAXCPW])
            iota_at = pc.tile([128, 128, ATSEGC], f16)
            nc.vector.tensor_copy(out=iota_at[:], in_=ii[:, :, 0:ATSEGC])
            startd_sb = pc.tile([128, NCOLAT], f16)
            nc.sync.dma_start(startd_sb[:], startd_d[:])
            mshift_sb = pc.tile([128, 2, 128], f16)
            nc.sync.dma_start(mshift_sb[:], mshift_d[:])

            ident = pc.tile([128, 128], f16)
            make_identity(nc, ident[:])

            ones32 = pc.tile([1, 128], f32)
            nc.vector.memset(ones32[:], 1.0)

            b1r = pc.tile([128, 64], f32)
            b2r = pc.tile([128, 40], f32)
            with tc.tile_pool(name="pini", bufs=2, space="PSUM") as ppi:
                for row_d, width, dest in ((b1, 64, b1r), (b2, 40, b2r)):
                    t = pc.tile([1, width], f32, tag=f"rrow{width}")
                    nc.sync.dma_start(t[:], row_d[:])
                    ps = ppi.tile([128, width], f32, tag="rep")
                    nc.tensor.matmul(ps[:], lhsT=ones32[:], rhs=t[:],
                                     start=True, stop=True)
                    nc.vector.tensor_copy(out=dest[:], in_=ps[:])

            # a_dst per-slot tiles, telescoped from the own-block a_dst
            # values via PE (P[d,s] = (s >= start[d]) is a step matrix;
            # P @ diff(ad) = ad[drel[s]] since slots are drel-sorted)
            at1_all = pc.tile([128, plan["NCHUNK"], 8], f16)
            at2_all = pc.tile([128, plan["NCHUNK"], 1], f16)
            ad2own = pc.tile([128, NW + 1, 1], f16)
            nc.vector.memset(ad2own[:, 0:1, :].rearrange("p a b -> p (a b)"),
                             0.0)

            def at_fill(dall, at_tile, H, pat, ppat):
                for (kg0, nk, c0, ncols, cols) in atsegs:
                    P = pat.tile([128, 128, ATSEGC], f16, tag="P")
                    st_ = startd_sb[:, c0:c0 + ncols]
                    nc.vector.tensor_tensor(
                        out=P[:, :, 0:ncols],
                        in0=iota_at[:, :, 0:ncols],
                        in1=BC(st_, [st_.ap[0], [0, 128], st_.ap[1]]),
                        op=OP.is_ge)
                    ps = ppat.tile([128, ATSEGC, 8], f32, tag="atps")
                    for (i, klocal, v, w, first, last) in cols:
                        nc.tensor.matmul(ps[:, klocal, 0:H],
                                         lhsT=P[:, :, i],
                                         rhs=dall[:, v, w, 0:H],
                                         start=first, stop=last)
                    nc.scalar.activation(out=at_tile[:, kg0:kg0 + nk, :],
                                         in_=ps[:, 0:nk, 0:H], func=AF.Copy)

            def build_dall(adown, H, dall, ppd):
                # adown: [128, NW+1, H] f16, col 0 zeroed.
                # dall[:,0,w,:] = fwd-diff (row d: ad[d]-ad[d-1], row0 ad[0]),
                # dall[:,1,w,:] = same but row0 = ad[0,w]-ad[127,w-1].
                # PSUM rows padded to whole banks (512 f32); each matmul's
                # output chunk must stay inside one bank.  The two diff
                # variants run sequentially through one 2-bank tile.
                psrow = ((NW * H + 511) // 512) * 512
                a_ = adown[:]
                for v in range(2):
                    ps = ppd.tile([128, psrow], f32, tag=f"dps{H}")
                    for off in range(0, NW * H, 512):
                        nn = min(512, NW * H - off)
                        rhs_cur = bass.AP(a_.tensor, a_.offset + H + off,
                                          [a_.ap[0], [1, nn]])
                        rhs_prev = bass.AP(a_.tensor, a_.offset + off,
                                           [a_.ap[0], [1, nn]])
                        nc.tensor.matmul(ps[:, off:off + nn],
                                         lhsT=mshift_sb[:, 0, :],
                                         rhs=rhs_cur, start=True,
                                         stop=(v == 0))
                        if v == 1:
                            nc.tensor.matmul(ps[:, off:off + nn],
                                             lhsT=mshift_sb[:, 1, :],
                                             rhs=rhs_prev, start=False,
                                             stop=True)
                    nc.scalar.activation(
                        out=dall[:, v, :, :].rearrange("p b c -> p (b c)"),
                        in_=ps[:, 0:NW * H], func=AF.Copy)

            # ---------- phase A (REPLICATED, no collective): the full-table
            # transform streams xTfull on DMA/PE/Act while, concurrently,
            # DVE runs the at1 telescoping.  ad1own = x_own @ v_d1 is
            # computed directly from the core's own x shard (same fused math
            # as the table's a_dst columns), so at1 never touches tab1;
            # engines are disjoint: DMA+PE+Act = transform, DVE+PE = at1.
            with (tc.tile_pool(name="pa", bufs=3) as pa,
                  tc.tile_pool(name="ppa", bufs=2, space="PSUM") as ppa,
                  tc.tile_pool(name="pat1", bufs=2) as pat1,
                  tc.tile_pool(name="ppat1", bufs=2, space="PSUM") as ppat1,
                  tc.tile_pool(name="ppd1", bufs=1, space="PSUM") as ppd1):
                # ad1own: per j-block matmul x_own[:, j] @ v_d1 (cols 72:80
                # of W1e), accumulated over the two k-halves
                ad1own = pat1.tile([128, NW + 1, 8], f16, tag="adown")
                nc.vector.memset(
                    ad1own[:, 0:1, :].rearrange("p a b -> p (a b)"), 0.0)
                JB = 10
                for j0 in range(0, NW, JB):
                    xo = pat1.tile([128, 2, JB * 128], f16, tag="xo")
                    for k in range(2):
                        nc.sync.dma_start(
                            xo[:, k, :],
                            xTloc[k, :, j0 * 128:(j0 + JB) * 128])
                    psad = ppat1.tile([128, JB, 8], f32, tag="psad")
                    for j in range(JB):
                        for k in range(2):
                            nc.tensor.matmul(
                                psad[:, j, :],
                                lhsT=xo[:, k, j * 128:(j + 1) * 128],
                                rhs=w1_sb[:, k, 72:80],
                                start=(k == 0), stop=(k == 1))
                    nc.scalar.copy(out=ad1own[:, 1 + j0:1 + j0 + JB, :],
                                   in_=psad[:])
                dall1 = pat1.tile([128, 2, NW, 8], f16, tag="dall")
                build_dall(ad1own, 8, dall1, ppd1)
                at_fill(dall1, at1_all, 8, pat1, ppat1)

                # replicated table transform (emitted after at1 so the DVE
                # queue holds only at1 work; evictions go to Act only)
                AB = 10                      # chunks per DMA batch
                for cb in range(8):
                    base = cb * NLOC
                    for jj in range(0, 100, AB):
                        nb = min(AB, 100 - jj)
                        xt = pa.tile([128, 2, AB * 128], f16, tag="xt")
                        for k in range(2):
                            nc.sync.dma_start(
                                xt[:, k, 0:nb * 128],
                                xTfull[k, :, base + jj * 128:
                                       base + (jj + nb) * 128])
                        row = pa.tile([128, AB, 128], f16, tag="row")
                        for u in range(0, nb, 4):
                            ub = min(4, nb - u)
                            ps = ppa.tile([128, 4, 80], f32, tag="np1")
                            for j in range(ub):
                                for k in range(2):
                                    nc.tensor.matmul(
                                        ps[:, j, :],
                                        lhsT=xt[:, k, (u + j) * 128:
                                                (u + j + 1) * 128],
                                        rhs=w1_sb[:, k, :], start=(k == 0),
                                        stop=(k == 1))
                            nc.scalar.copy(out=row[:, u:u + ub, 0:80],
                                           in_=ps[:, 0:ub, :])
                        nc.sync.dma_start(
                            bass.AP(tab1, (base + jj) * 128,
                                    [[NW * 128, 128], [256, nb // 2],
                                     [1, 256]]),
                            row[:, 0:nb, :])

            # ---------- phase B: layer-1 edge pass ----------
            def edge_pass(layer):
                if layer == 1:
                    tab, ncols_h, as_col = tab1, 72, 64
                    nheads, msgw = 8, 72
                else:
                    tab, ncols_h, as_col = ago, 41, 40
                    nheads, msgw = 1, 41
                pool_name = f"pe{layer}"
                with (tc.tile_pool(name=pool_name, bufs=2) as pb,
                      tc.tile_pool(name=pool_name + "h", bufs=2) as ph,
                      tc.tile_pool(name=pool_name + "m", bufs=1) as pm,
                      tc.tile_pool(name=pool_name + "w",
                                   bufs=(2 if layer == 1 else 3)) as pw,
                      tc.tile_pool(name=pool_name + "p", bufs=2,
                                   space="PSUM") as ppb):
                    qn = 0
                    for s in range(NSB):
                        k0, k1 = sb_chunks[s]
                        cps = k1 - k0
                        hix = pb.tile([128, CPSB_MAX * 8], i16, tag="hix")
                        nc.sync.dma_start(hix[:, 0:cps * 8],
                                          hidx_d[:, k0 * 8:k1 * 8])

                        ht = ph.tile([128, CPSB_MAX, ncols_h], f16, tag="ht")
                        for (ss, q, slot0, nids) in plan["hcalls"]:
                            if ss != s:
                                continue
                            c0 = slot0 // 128 - k0
                            _gather_small(
                                nc.gpsimd,
                                ht[:, c0:c0 + nids // 128, :],
                                tab[q * QS:(q + 1) * QS, 0:ncols_h],
                                hix[:, (slot0 - k0 * 128) // 16:
                                    (slot0 - k0 * 128 + nids) // 16],
                                nids, ncols_h, 128, queue_num=qn % 4)
                            qn += 1
                        if layer == 1:
                            at_s = at1_all[:, k0:k1, :]
                        else:
                            at_s = at2_all[:, k0:k1, 0:1]

                        e = pm.tile([128, CPSB_MAX, nheads], f32, tag="e")
                        lr = e
                        nc.vector.tensor_tensor(
                            out=e[:, 0:cps, :],
                            in0=ht[:, 0:cps, as_col:as_col + nheads],
                            in1=at_s, op=OP.add)
                        nc.vector.scalar_tensor_tensor(
                            out=lr[:, 0:cps, :], in0=e[:, 0:cps, :],
                            scalar=0.2, in1=e[:, 0:cps, :],
                            op0=OP.mult, op1=OP.max)

                        msg = pm.tile([128, CPSB_MAX, msgw], bf16, tag="msg")
                        # w into msg's trailing cols (compact exp)
                        nc.scalar.activation(
                            out=msg[:, 0:cps, as_col:as_col + nheads],
                            in_=lr[:, 0:cps, :], func=AF.Exp)
                        if layer == 1:
                            # expanded weights for a clean 2x-mode mult
                            half = (CPSB_MAX + 1) // 2
                            wgx = pm.tile([128, half, 8, 8], bf16, tag="wgx")
                            for h0 in (0, half):
                                hn = min(half, cps - h0)
                                if hn <= 0:
                                    continue
                                lrs = lr[:, h0:h0 + hn, :]
                                nc.scalar.activation(
                                    out=wgx[:, 0:hn, :, :],
                                    in_=BC(lrs, [lrs.ap[0], lrs.ap[1],
                                                 lrs.ap[2], [0, 8]]),
                                    func=AF.Exp)
                                m_ = msg[:, h0:h0 + hn, 0:64]
                                h_ = ht[:, h0:h0 + hn, 0:64]
                                nc.vector.tensor_tensor(
                                    out=BC(m_, [m_.ap[0], m_.ap[1],
                                                [8, 8], [1, 8]]),
                                    in0=BC(h_, [h_.ap[0], h_.ap[1],
                                                [8, 8], [1, 8]]),
                                    in1=wgx[:, 0:hn, :, :], op=OP.mult)
                        else:
                            wgx2 = pw.tile([128, CPSB_MAX, 40], bf16,
                                           tag="wgx2")
                            lrs = lr[:, 0:cps, :]
                            nc.scalar.activation(
                                out=wgx2[:, 0:cps, :],
                                in_=BC(lrs, [lrs.ap[0], lrs.ap[1], [0, 40]]),
                                func=AF.Exp)
                            nc.vector.tensor_tensor(
                                out=msg[:, 0:cps, 0:40],
                                in0=ht[:, 0:cps, 0:40],
                                in1=wgx2[:, 0:cps, :], op=OP.mult)

                        # windows: one-hot + aggregation matmuls, PSUM
                        # evicted into a per-sb batch tile
                        hsb = pm.tile([128, NWSB, msgw], f32, tag="hsb")
                        for wi in range(NWSB):
                            w = s * NWSB + wi
                            cols = wcols[w]
                            cpw = len(cols)
                            c0 = colbase[w]
                            ohT = pw.tile([128, 128, MAXCPW], bf16, tag="ohT")
                            dr = drel_sb[:, c0:c0 + cpw]
                            nc.vector.tensor_tensor(
                                out=ohT[:, :, 0:cpw],
                                in0=BC(dr, [dr.ap[0], [0, 128], dr.ap[1]]),
                                in1=iota_rep[:, :, 0:cpw], op=OP.is_equal)
                            ps = ppb.tile([128, msgw], f32, tag="agg")
                            for i, k in enumerate(cols):
                                nc.tensor.matmul(
                                    ps[:], lhsT=ohT[:, :, i],
                                    rhs=msg[:, k - k0, :],
                                    start=(i == 0), stop=(i == cpw - 1))
                            nc.scalar.copy(out=hsb[:, wi, :], in_=ps[:])

                        # per-sb batched softmax-normalize (+ elu/r2 for L1)
                        if layer == 1:
                            den = pw.tile([128, NWSB, 8], f32, tag="den")
                            nc.scalar.activation(out=den[:],
                                                 in_=hsb[:, :, 64:72],
                                                 func=AF.Copy, bias=ACC_EPS)
                            rec = pw.tile([128, NWSB, 8], f32, tag="rec")
                            nc.vector.reciprocal(
                                rec[:].rearrange("p a b -> p (a b)"),
                                den[:].rearrange("p a b -> p (a b)"))
                            o1 = pw.tile([128, NWSB, 64], f32, tag="o1")
                            nu = hsb[:, :, 0:64]
                            r_ = rec[:]
                            nc.vector.tensor_tensor(
                                out=BC(o1[:], [o1[:].ap[0], [64, NWSB],
                                               [8, 8], [1, 8]]),
                                in0=BC(nu, [nu.ap[0], [72, NWSB],
                                            [8, 8], [1, 8]]),
                                in1=BC(r_, [r_.ap[0], [8, NWSB],
                                            [1, 8], [0, 8]]),
                                op=OP.mult)
                            o1v = o1[:].rearrange("p a b -> p (a b)")
                            if not plan["skip_b1"]:
                                b1w = b1r[:]
                                nc.vector.tensor_tensor(
                                    out=o1v,
                                    in0=o1v,
                                    in1=BC(b1w, [b1w.ap[0], [0, NWSB],
                                                 [1, 64]]),
                                    op=OP.add)
                            # elu = relu(x) + exp(-relu(-x)) - 1
                            rneg = pw.tile([128, NWSB, 64], f32, tag="rneg")
                            nc.scalar.activation(
                                out=rneg[:].rearrange("p a b -> p (a b)"),
                                in_=o1v, func=AF.Relu, scale=-1.0)
                            expn = rneg
                            nc.scalar.activation(
                                out=expn[:].rearrange("p a b -> p (a b)"),
                                in_=rneg[:].rearrange("p a b -> p (a b)"),
                                func=AF.Exp, scale=-1.0)
                            pos = pw.tile([128, NWSB, 64], f32, tag="pos")
                            nc.scalar.activation(
                                out=pos[:].rearrange("p a b -> p (a b)"),
                                in_=o1v, func=AF.Relu)
                            hl16 = pw.tile([128, NWSB, 64], f16, tag="hl16")
                            nc.vector.scalar_tensor_tensor(
                                out=hl16[:].rearrange("p a b -> p (a b)"),
                                in0=expn[:].rearrange("p a b -> p (a b)"),
                                scalar=-1.0,
                                in1=pos[:].rearrange("p a b -> p (a b)"),
                                op0=OP.add, op1=OP.add)
                            r2s = pw.tile([128, NWSB, 42], f16, tag="r2s")
                            for wi in range(NWSB):
                                pst = ppb.tile([64, 128], f16, tag="tr")
                                nc.tensor.transpose(out=pst[:],
                                                    in_=hl16[:, wi, :],
                                                    identity=ident[:])
                                hlT = pw.tile([64, 128], f16, tag="hlT")
                                nc.scalar.copy(out=hlT[:], in_=pst[:])
                                r2p = ppb.tile([128, 42], f32, tag="r2p")
                                nc.tensor.matmul(r2p[:], lhsT=hlT[:],
                                                 rhs=w2_sb[:], start=True,
                                                 stop=True)
                                nc.scalar.copy(out=r2s[:, wi, :], in_=r2p[:])
                            nc.sync.dma_start(
                                bass.AP(agi2, (s * NWSB) * 128,
                                        [[NW * 128, 128], [128, NWSB],
                                         [1, 42]]),
                                r2s[:])
                            # stash a_dst2 (col 41) for the L2 telescoping
                            nc.scalar.copy(
                                out=ad2own[:, 1 + s * NWSB:
                                           1 + (s + 1) * NWSB, :],
                                in_=r2s[:, :, 41:42])
                        else:
                            den = pw.tile([128, NWSB, 1], f32, tag="den2")
                            nc.scalar.activation(out=den[:],
                                                 in_=hsb[:, :, 40:41],
                                                 func=AF.Copy, bias=ACC_EPS)
                            rec = pw.tile([128, NWSB, 1], f32, tag="rec2")
                            nc.vector.reciprocal(
                                rec[:].rearrange("p a b -> p (a b)"),
                                den[:].rearrange("p a b -> p (a b)"))
                            o2 = pw.tile([128, NWSB, 40], f32, tag="o2")
                            nu = hsb[:, :, 0:40]
                            r_ = rec[:]
                            nc.vector.tensor_tensor(
                                out=o2[:],
                                in0=BC(nu, [nu.ap[0], [41, NWSB], [1, 40]]),
                                in1=BC(r_, [r_.ap[0], [1, NWSB], [0, 40]]),
                                op=OP.mult)
                            o2v = o2[:].rearrange("p a b -> p (a b)")
                            if not plan["skip_b2"]:
                                b2w = b2r[:]
                                nc.vector.tensor_tensor(
                                    out=o2v, in0=o2v,
                                    in1=BC(b2w, [b2w.ap[0], [0, NWSB],
                                                 [1, 40]]),
                                    op=OP.add)
                            nc.sync.dma_start(
                                bass.AP(out, (s * NWSB) * 128 * 40,
                                        [[40, 128], [128 * 40, NWSB],
                                         [1, 40]]),
                                o2[:])

            edge_pass(1)

            # ---------- AllGather first; the at2 telescoping (local data
            # only) runs during the collective ----------
            nc.gpsimd.collective_compute(
                "AllGather", OP.bypass, ins=[agi2[:]], outs=[ago[:]],
                replica_groups=[list(range(8))])

            with (tc.tile_pool(name="pat2", bufs=2) as pat2,
                  tc.tile_pool(name="ppat2", bufs=2, space="PSUM") as ppat2,
                  tc.tile_pool(name="ppd2", bufs=1, space="PSUM") as ppd2):
                dall2 = pat2.tile([128, 2, NW, 1], f16, tag="dall")
                build_dall(ad2own, 1, dall2, ppd2)
                at_fill(dall2, at2_all, 1, pat2, ppat2)

            edge_pass(2)

    nc.finalize()
    return nc


def kernel(**inputs):
    per_core, plan = _host_prep(**inputs)
    if "nc" not in _CACHE:
        _CACHE["nc"] = _build_nc(plan)
    nc = _CACHE["nc"]
    from concourse.bass_utils import run_bass_kernel_spmd
    res = run_bass_kernel_spmd(nc, per_core, list(range(8)))
    full = np.concatenate([res.results[c]["out"] for c in range(8)], axis=0)
    return np.ascontiguousarray(full[:N]).astype(np.float32)



# revision 74
# speedup vs baseline: 1.0456x; 1.0065x over previous
"""GAT (2-layer PyG GATConv, eval) on 8 Trainium2 NeuronCores.

Sharding: nodes range-partitioned (NLOC=12800/core); core c owns edges whose
dst is in its range. The layer-1 node table is REPLICATED: every core
transforms all 102400 rows from the full x (the x stream + table write cost
less on the DMA timeline than the AllGather they replace, and the a_dst
telescoping runs concurrently on DVE from the core's own x shard). Layer 2's
table still needs one AllGather (agi2 -> ago), overlapped by the at2
telescoping machinery.

Slot layout per core: superblock (10 windows) -> quadrant -> window, with
per-(window,quadrant) STATIC capacities = max edge count over the 8 cores
(SPMD: one module runs on all cores; only tensor contents differ) — ~10%
slot padding vs 28% for fixed-size groups. Gather calls use
single_packet=False, which lets the SWDGE ucode stream descriptors through
the ring: up to 4x the ring (dynamic_dma_scratch_size/16) indices per call
(probed on HW: 8192 idx streams through a 2048-desc ring; 8x crashes;
single-packet calls hard-crash above 1024 idx). One ~5.5k-idx call per
(sb,quadrant) segment cuts the 994ns-per-call Pool overhead ~5x vs the
1024-idx baseline.

Per layer, per edge slot: a 144B/82B payload gather pulls [h|a_src] rows
(256B-stride tables, int16 idx into 25600-row quadrants). a_dst is NOT
gathered: slots are drel-sorted inside each (window,quadrant) group, so
per-slot a_dst[drel] telescopes — P[d,s] = (s >= start[d]) is a DVE is_ge
step matrix (host-static start tables) and PE computes P^T @ diff(a_dst)
per chunk column; window starts mid-chunk use a compensated diff table
(row0 = ad[0,w]-ad[127,w-1]) accumulated in the same PSUM. Both at1 and at2
expansions run during their layer's AllGather (they read only local data).
Softmax is the shift-invariant no-max form (w = exp(leakyrelu(as+ad)),
|e| < ~25 so fp32 exp is safe). Segment reduction is a PE matmul whose
stationary matrix is a transposed one-hot built by DVE is_equal in the
2x-mode layout; boundary chunks carry a masked drel column per touching
window. msg = h*w uses an Act-expanded weight tile so the DVE mult runs in
2x mode. Softmax normalize + ELU + the r2 = hlT @ [W2|a2_src|a2_dst]
projection are batched per superblock; tables are written node-permuted
(row = p*100 + j within each core block) so writes coalesce per partition.
"""
import numpy as np
import ml_dtypes

N = 100000
E = 1600000
NF = 256
HEADS, NHID = 8, 8
NH = HEADS * NHID          # 64
NCLASS = 40
NLOC = 12800               # nodes per core
NW = 100                   # 128-dst windows per core
NQ = 4                     # src table quadrants
QS = 25600                 # rows per quadrant
NWSB = 10                  # windows per superblock
NSB = NW // NWSB           # 10 superblocks
NTOT = 102400
ACC_EPS = 1e-16

_CACHE = {}


def _ceil128(x):
    return (x + 127) & ~127


def _host_prep(x, edge_index, W1, a1_src, a1_dst, b1, W2, a2_src, a2_dst, b2):
    src = np.asarray(edge_index[0], dtype=np.int64)
    dst = np.asarray(edge_index[1], dtype=np.int64)

    # table-row permutation: node n -> row  c*NLOC + (l%128)*NW + l//128
    def rowperm(n):
        c = n // NLOC
        l = n - c * NLOC
        return c * NLOC + (l % 128) * NW + l // 128

    srow = rowperm(src)
    sq = srow // QS
    sidx = (srow - sq * QS).astype(np.int16)

    core = dst // NLOC
    dloc = dst - core * NLOC
    w_e = (dloc >> 7).astype(np.int64)
    dr_e = (dloc & 127).astype(np.int64)
    adidx = (dr_e * NW + w_e).astype(np.int16)
    sb_e = w_e // NWSB

    # static capacities: max over cores per (window, quadrant); >=1 so every
    # group is present in the slot stream (the telescoped a_dst expansion
    # needs window w-1 to precede window w inside each (sb,q) segment)
    gkey = (core * NW + w_e) * NQ + sq          # [E]
    cnt = np.bincount(gkey, minlength=8 * NW * NQ).reshape(8, NW, NQ)
    cap = np.maximum(cnt.max(axis=0), 1)         # [NW, NQ]

    # slot layout: sb -> quadrant -> window.  With single_packet=False the
    # SWDGE gather ucode streams descriptors through the ring, so calls up
    # to 4x the ring size (dynamic_dma_scratch_size/16) are fine (probed on
    # HW: 8192 idx streams through a 2048-desc ring; 8x crashes).
    MAXIDX = 8192
    wq_start = np.zeros((NW, NQ), np.int64)
    hcalls = []                                  # (sb, q, slot0, n_idx)
    sb_chunks = []                               # (k0, k1) per sb
    nslot = 0
    for s in range(NSB):
        k0 = nslot // 128
        for q in range(NQ):
            seg0 = nslot
            for w in range(s * NWSB, (s + 1) * NWSB):
                wq_start[w, q] = nslot
                nslot += int(cap[w, q])
            nslot = _ceil128(nslot)
            for off in range(seg0, nslot, MAXIDX):
                hcalls.append((s, q, off, min(MAXIDX, nslot - off)))
        sb_chunks.append((k0, nslot // 128))
    NSLOT = nslot
    NCHUNK = NSLOT // 128

    # window label per slot (shared across cores: layout is static).  Group
    # pads inherit their group's window; (sb,q)-tail ceil128 pads inherit the
    # segment's last window.
    wfull = np.zeros(NSLOT, np.int64)
    for s in range(NSB):
        for q in range(NQ):
            for w in range(s * NWSB, (s + 1) * NWSB):
                a = int(wq_start[w, q])
                e_ = a + int(cap[w, q])
                wfull[a:e_] = w
            wfull[e_:_ceil128(e_)] = (s + 1) * NWSB - 1   # segment tail pads

    # at-plan: per chunk, one column-copy per touching window; copies of one
    # chunk accumulate in PSUM.  v=1 (compensated diff table) iff the window
    # starts mid-chunk.  Packed into segments of <= ATSEGC columns.
    ATSEGC = 24
    atsegs = []            # (kg0, nk, c0, cols=[(i, klocal, v, w, first, last)])
    copies_per_chunk = []  # [(w, a, b)]
    for k in range(NCHUNK):
        wk = wfull[k * 128:(k + 1) * 128]
        runs = []
        a = 0
        for p in range(1, 128):
            if wk[p] != wk[p - 1]:
                runs.append((int(wk[a]), a, p))
                a = p
        runs.append((int(wk[a]), a, 128))
        copies_per_chunk.append(runs)
    ncolat = 0
    cur = None
    colat_of = {}
    for k in range(NCHUNK):
        runs = copies_per_chunk[k]
        if cur is None or cur[3] + len(runs) > ATSEGC:
            if cur is not None:
                atsegs.append(cur)
            cur = [k, 0, ncolat, 0, []]
        kloc = cur[1]
        for i, (w, a, b) in enumerate(runs):
            colat_of[(k, a)] = ncolat
            cur[4].append((cur[3], kloc, 1 if a > 0 else 0, w,
                           i == 0, i == len(runs) - 1))
            cur[3] += 1
            ncolat += 1
        cur[1] += 1
    if cur is not None:
        atsegs.append(cur)
    NCOLAT = ncolat

    # per-window chunk columns
    colmap = np.full((NW, NCHUNK), -1, np.int64)
    wcols = []                                   # per w: (colbase, [chunks])
    ncol = 0
    for w in range(NW):
        cols = []
        for q in range(NQ):
            a = int(wq_start[w, q])
            b = a + int(cap[w, q])
            for k in range(a // 128, (b + 127) // 128):
                cols.append(k)
                colmap[w, k] = ncol
                ncol += 1
        wcols.append(cols)
    NCOL = ncol
    MAXCPW = max(len(c) for c in wcols)

    plan = {
        "NSLOT": NSLOT, "NCHUNK": NCHUNK, "NCOL": NCOL, "MAXCPW": MAXCPW,
        "NCOLAT": NCOLAT, "ATSEGC": ATSEGC,
        "hcalls": hcalls, "atsegs": atsegs, "sb_chunks": sb_chunks,
        "wcols": wcols,
        "skip_b1": bool(np.all(np.asarray(b1) == 0)),
        "skip_b2": bool(np.all(np.asarray(b2) == 0)),
    }

    # group-id in slot order: (sb, q, w_in_sb)
    flatg = (sb_e * NQ + sq) * NWSB + (w_e - sb_e * NWSB)
    gstart_flat = np.zeros(NSB * NQ * NWSB, np.int64)
    for s in range(NSB):
        for q in range(NQ):
            for wi in range(NWSB):
                gstart_flat[(s * NQ + q) * NWSB + wi] = wq_start[s * NWSB + wi, q]

    per_core = []
    hidx_all, startd_all, drel_all = [], [], []
    for c in range(8):
        m = core == c
        fg = flatg[m]
        drc = dr_e[m]
        order = np.lexsort((drc, fg))
        fgs = fg[order]
        cntc = np.bincount(fgs, minlength=NSB * NQ * NWSB)
        starts = np.zeros_like(cntc)
        starts[1:] = np.cumsum(cntc)[:-1]
        rank = np.arange(len(fgs)) - starts[fgs]
        slot = gstart_flat[fgs] + rank

        hvec = np.zeros(NSLOT, np.int16)
        hvec[slot] = sidx[m][order]

        drel = np.full((128, NCOL), 128.0, np.float32)
        k_s = slot >> 7
        p_s = slot & 127
        we_s = w_e[m][order]
        col_s = colmap[we_s, k_s]
        assert (col_s >= 0).all()
        drel[p_s, col_s] = drc[order].astype(np.float32)

        # per-slot drel stream (pads = 128) for the telescoped start tables
        drfull = np.full(NSLOT, 128, np.int64)
        drfull[slot] = drc[order]
        startd = np.zeros((128, NCOLAT), np.float32)
        dgrid = np.arange(128)
        for k in range(NCHUNK):
            for (w, a, b) in copies_per_chunk[k]:
                col = colat_of[(k, a)]
                drs = drfull[k * 128 + a:k * 128 + b]
                startd[:, col] = a + np.searchsorted(drs, dgrid)

        def wrap16(v):
            o = np.zeros((128, NSLOT // 16), np.int16)
            sl = np.arange(NSLOT)
            o[sl % 16, sl // 16] = v
            for r in range(1, 8):
                o[16 * r:16 * (r + 1)] = o[:16]
            return o

        hidx_all.append(wrap16(hvec))
        startd_all.append(startd.astype(np.float16))
        drel_all.append(drel.astype(ml_dtypes.bfloat16))

    # weights
    W1 = np.asarray(W1, np.float32)
    v_s1 = np.einsum("chk,hk->ch", W1.reshape(NF, HEADS, NHID),
                     np.asarray(a1_src, np.float32))
    v_d1 = np.einsum("chk,hk->ch", W1.reshape(NF, HEADS, NHID),
                     np.asarray(a1_dst, np.float32))
    W1e = np.concatenate([W1, v_s1, v_d1], axis=1).reshape(2, 128, 80)
    W1e = W1e.astype(np.float16)

    W2 = np.asarray(W2, np.float32)
    v_s2 = W2 @ np.asarray(a2_src, np.float32)[0]
    v_d2 = W2 @ np.asarray(a2_dst, np.float32)[0]
    W2e = np.concatenate([W2, v_s2[:, None], v_d2[:, None]],
                         axis=1).astype(np.float16)   # [64, 42]

    xp = np.zeros((NTOT, NF), np.float32)
    xp[:N] = np.asarray(x, np.float32)

    # lhsT matrices for building the diff tables on PE:
    # mshift[:,0,:] = Mplain^T (fwd diff), mshift[:,1,:] = -sel(127)->row0
    mshiftT = np.zeros((128, 2, 128), np.float16)
    mshiftT[:, 0, :] = (np.eye(128) - np.eye(128, k=1)).astype(np.float16)
    mshiftT[127, 1, 0] = -1.0

    # full transposed x, shared by all cores: the layer-1 table transform is
    # replicated (each core computes all 102400 rows locally; the x stream +
    # table write fit under what the AllGather used to cost, and the a_dst
    # telescoping runs concurrently from the core's own x shard)
    xT = np.ascontiguousarray(xp.T).astype(np.float16).reshape(2, 128, NTOT)

    for c in range(8):
        xloc = np.ascontiguousarray(xp[c * NLOC:(c + 1) * NLOC].T)
        per_core.append({
            "xTloc": xloc.astype(np.float16).reshape(2, 128, NLOC),
            "xTfull": xT,
            "W1e": W1e,
            "W2e": W2e,
            "b1": np.asarray(b1, np.float32)[None, :],
            "b2": np.asarray(b2, np.float32)[None, :],
            "hidx": hidx_all[c],
            "startd": startd_all[c],
            "drel": drel_all[c],
            "mshift": mshiftT,
        })
    return per_core, plan


def _gather_small(g, out_ap, in_ap, idxs_ap, num_idxs, elem_size, elem_step,
                  queue_num=0, single_packet=False):
    """dma_gather with payload < 256B; only the 256B row-stride rule is real
    for the non-transpose path."""
    import concourse.mybir as mybir
    stride_bytes = elem_step * mybir.dt.size(in_ap.dtype)
    assert stride_bytes % 256 == 0
    _in_ap = g.lower_ap_dma(in_ap, for_custom_bir_dma=True)
    _idxs_ap = g.lower_ap(idxs_ap)
    _out_ap = g.lower_ap(out_ap)
    return g.add_instruction(mybir.InstDMAGatherAnt(
        name=g.bass.get_next_instruction_name(),
        ins=[*_in_ap, _idxs_ap, g.lower_val_access(g.to_reg(num_idxs))],
        outs=[_out_ap],
        transpose=False,
        num_idxs=num_idxs,
        elem_size=elem_size,
        stride_bytes_256=stride_bytes // 256,
        gen_mode=0,
        single_packet=single_packet,
        queue_num=queue_num,
        sbuf_tokens_per_rank=0,
        sbuf_free_dim_per_rank=0,
        sbuf_free_dim_pad_per_rank=0,
        sbuf_byte_offset=0,
    ))


def _build_nc(plan):
    import concourse.bass as bass
    import concourse.bacc as bacc
    import concourse.mybir as mybir
    import concourse.tile as tile
    from concourse.library_config import mlp
    from concourse.masks import make_identity

    f32, f16, bf16, i16 = (mybir.dt.float32, mybir.dt.float16,
                           mybir.dt.bfloat16, mybir.dt.int16)
    AF = mybir.ActivationFunctionType
    OP = mybir.AluOpType

    NSLOT = plan["NSLOT"]
    NCOL = plan["NCOL"]
    MAXCPW = plan["MAXCPW"]
    sb_chunks = plan["sb_chunks"]
    wcols = plan["wcols"]
    CPSB_MAX = max(k1 - k0 for k0, k1 in sb_chunks)
    colbase = [0] * NW
    for w in range(1, NW):
        colbase[w] = colbase[w - 1] + len(wcols[w - 1])

    NCOLAT = plan["NCOLAT"]
    ATSEGC = plan["ATSEGC"]
    atsegs = plan["atsegs"]

    nc = bacc.Bacc("TRN2", target_bir_lowering=False, debug=False,
                   num_devices=8, num_swdge_queues=4,
                   dynamic_dma_scratch_size=32768)

    xTloc = nc.dram_tensor("xTloc", [2, 128, NLOC], f16, kind="ExternalInput")
    xTfull = nc.dram_tensor("xTfull", [2, 128, NTOT], f16,
                            kind="ExternalInput")
    W1e = nc.dram_tensor("W1e", [2, 128, 80], f16, kind="ExternalInput")
    W2e = nc.dram_tensor("W2e", [64, 42], f16, kind="ExternalInput")
    b1 = nc.dram_tensor("b1", [1, 64], f32, kind="ExternalInput")
    b2 = nc.dram_tensor("b2", [1, 40], f32, kind="ExternalInput")
    hidx_d = nc.dram_tensor("hidx", [128, NSLOT // 16], i16,
                            kind="ExternalInput")
    startd_d = nc.dram_tensor("startd", [128, NCOLAT], f16,
                              kind="ExternalInput")
    mshift_d = nc.dram_tensor("mshift", [128, 2, 128], f16,
                              kind="ExternalInput")
    drel_d = nc.dram_tensor("drel", [128, NCOL], bf16, kind="ExternalInput")
    out = nc.dram_tensor("out", [NLOC, 40], f32, kind="ExternalOutput")

    # per-QUADRANT table tensors: (sb,q) gathers depend only on quadrant q's
    # transform writes, so the gather stream overlaps the transform tail
    tab1q = [nc.dram_tensor(f"tab1q{q}", [QS, 128], f16) for q in range(4)]
    agi2 = nc.dram_tensor("agi2", [NLOC, 128], f16)    # [h2|as2|ad2|pad]
    ago = nc.dram_tensor("ago", [NTOT, 128], f16, addr_space="Shared")

    def BC(ap, dims):
        return bass.AP(ap.tensor, ap.offset, dims)

    def dram_rows(t, offset_rows, dims):
        """AP into DRAM tensor t (row-major, 128 f16 cols) at row offset."""
        return bass.AP(t, offset_rows * 128, dims)

    with tile.TileContext(nc) as tc:
        with tc.tile_pool(name="const", bufs=1) as pc:
            nc.gpsimd.load_library(mlp)

            drel_sb = pc.tile([128, NCOL], bf16)
            nc.sync.dma_start(drel_sb[:], drel_d[:])
            w1_sb = pc.tile([128, 2, 80], f16)
            nc.sync.dma_start(w1_sb[:], W1e[:].rearrange("k p n -> p k n"))
            w2_sb = pc.tile([64, 42], f16)
            nc.sync.dma_start(w2_sb[:], W2e[:])

            NIOTA = max(MAXCPW, ATSEGC)
            ii = pc.tile([128, 128, NIOTA], i16)
            nc.gpsimd.iota(ii[:], pattern=[[1, 128], [0, NIOTA]], base=0,
                           channel_multiplier=0)
            iota_rep = pc.tile([128, 128, MAXCPW], bf16)
            nc.vector.tensor_copy(out=iota_rep[:], in_=ii[:, :, 0:MAXCPW])
            iota_at = pc.tile([128, 128, ATSEGC], f16)
            nc.vector.tensor_copy(out=iota_at[:], in_=ii[:, :, 0:ATSEGC])
            startd_sb = pc.tile([128, NCOLAT], f16)
            nc.sync.dma_start(startd_sb[:], startd_d[:])
            mshift_sb = pc.tile([128, 2, 128], f16)
            nc.sync.dma_start(mshift_sb[:], mshift_d[:])

            ident = pc.tile([128, 128], f16)
            make_identity(nc, ident[:])

            ones32 = pc.tile([1, 128], f32)
            nc.vector.memset(ones32[:], 1.0)

            b1r = pc.tile([128, 64], f32)
            b2r = pc.tile([128, 40], f32)
            with tc.tile_pool(name="pini", bufs=2, space="PSUM") as ppi:
                for row_d, width, dest in ((b1, 64, b1r), (b2, 40, b2r)):
                    t = pc.tile([1, width], f32, tag=f"rrow{width}")
                    nc.sync.dma_start(t[:], row_d[:])
                    ps = ppi.tile([128, width], f32, tag="rep")
                    nc.tensor.matmul(ps[:], lhsT=ones32[:], rhs=t[:],
                                     start=True, stop=True)
                    nc.vector.tensor_copy(out=dest[:], in_=ps[:])

            # a_dst per-slot tiles, telescoped from the own-block a_dst
            # values via PE (P[d,s] = (s >= start[d]) is a step matrix;
            # P @ diff(ad) = ad[drel[s]] since slots are drel-sorted)
            at1_all = pc.tile([128, plan["NCHUNK"], 8], f16)
            at2_all = pc.tile([128, plan["NCHUNK"], 1], f16)
            ad2own = pc.tile([128, NW + 1, 1], f16)
            nc.vector.memset(ad2own[:, 0:1, :].rearrange("p a b -> p (a b)"),
                             0.0)

            def at_fill(dall, at_tile, H, pat, ppat):
                for (kg0, nk, c0, ncols, cols) in atsegs:
                    P = pat.tile([128, 128, ATSEGC], f16, tag="P")
                    st_ = startd_sb[:, c0:c0 + ncols]
                    nc.vector.tensor_tensor(
                        out=P[:, :, 0:ncols],
                        in0=iota_at[:, :, 0:ncols],
                        in1=BC(st_, [st_.ap[0], [0, 128], st_.ap[1]]),
                        op=OP.is_ge)
                    ps = ppat.tile([128, ATSEGC, 8], f32, tag="atps")
                    for (i, klocal, v, w, first, last) in cols:
                        nc.tensor.matmul(ps[:, klocal, 0:H],
                                         lhsT=P[:, :, i],
                                         rhs=dall[:, v, w, 0:H],
                                         start=first, stop=last)
                    nc.scalar.activation(out=at_tile[:, kg0:kg0 + nk, :],
                                         in_=ps[:, 0:nk, 0:H], func=AF.Copy)

            def build_dall(adown, H, dall, ppd):
                # adown: [128, NW+1, H] f16, col 0 zeroed.
                # dall[:,0,w,:] = fwd-diff (row d: ad[d]-ad[d-1], row0 ad[0]),
                # dall[:,1,w,:] = same but row0 = ad[0,w]-ad[127,w-1].
                # PSUM rows padded to whole banks (512 f32); each matmul's
                # output chunk must stay inside one bank.  The two diff
                # variants run sequentially through one 2-bank tile.
                psrow = ((NW * H + 511) // 512) * 512
                a_ = adown[:]
                for v in range(2):
                    ps = ppd.tile([128, psrow], f32, tag=f"dps{H}")
                    for off in range(0, NW * H, 512):
                        nn = min(512, NW * H - off)
                        rhs_cur = bass.AP(a_.tensor, a_.offset + H + off,
                                          [a_.ap[0], [1, nn]])
                        rhs_prev = bass.AP(a_.tensor, a_.offset + off,
                                           [a_.ap[0], [1, nn]])
                        nc.tensor.matmul(ps[:, off:off + nn],
                                         lhsT=mshift_sb[:, 0, :],
                                         rhs=rhs_cur, start=True,
                                         stop=(v == 0))
                        if v == 1:
                            nc.tensor.matmul(ps[:, off:off + nn],
                                             lhsT=mshift_sb[:, 1, :],
                                             rhs=rhs_prev, start=False,
                                             stop=True)
                    nc.scalar.activation(
                        out=dall[:, v, :, :].rearrange("p b c -> p (b c)"),
                        in_=ps[:, 0:NW * H], func=AF.Copy)

            # Allocate the L1 gather pools BEFORE (under) the transform pools
            # in the stack allocator: address-disjoint tiles carry no WAR
            # dep on the transform scope, and with per-quadrant table
            # tensors the (sb,q) gathers start as soon as quadrant q lands.
            import contextlib
            estack = contextlib.ExitStack()
            pb1 = estack.enter_context(tc.tile_pool(name="pe1", bufs=2))
            ph1 = estack.enter_context(tc.tile_pool(name="pe1h", bufs=2))

            l1pre = {}

            # ---------- phase A (REPLICATED, no collective): the full-table
            # transform streams xTfull on DMA/PE/Act while, concurrently,
            # DVE runs the at1 telescoping.  ad1own = x_own @ v_d1 is
            # computed directly from the core's own x shard (same fused math
            # as the table's a_dst columns), so at1 never touches tab1;
            # engines are disjoint: DMA+PE+Act = transform, DVE+PE = at1.
            with (tc.tile_pool(name="pa", bufs=3) as pa,
                  tc.tile_pool(name="ppa", bufs=2, space="PSUM") as ppa,
                  tc.tile_pool(name="pat1", bufs=2) as pat1,
                  tc.tile_pool(name="ppat1", bufs=2, space="PSUM") as ppat1,
                  tc.tile_pool(name="ppd1", bufs=1, space="PSUM") as ppd1):
                # ad1own: per j-block matmul x_own[:, j] @ v_d1 (cols 72:80
                # of W1e), accumulated over the two k-halves
                ad1own = pat1.tile([128, NW + 1, 8], f16, tag="adown")
                nc.vector.memset(
                    ad1own[:, 0:1, :].rearrange("p a b -> p (a b)"), 0.0)
                JB = 10
                for j0 in range(0, NW, JB):
                    xo = pat1.tile([128, 2, JB * 128], f16, tag="xo")
                    for k in range(2):
                        nc.sync.dma_start(
                            xo[:, k, :],
                            xTloc[k, :, j0 * 128:(j0 + JB) * 128])
                    psad = ppat1.tile([128, JB, 8], f32, tag="psad")
                    for j in range(JB):
                        for k in range(2):
                            nc.tensor.matmul(
                                psad[:, j, :],
                                lhsT=xo[:, k, j * 128:(j + 1) * 128],
                                rhs=w1_sb[:, k, 72:80],
                                start=(k == 0), stop=(k == 1))
                    nc.scalar.copy(out=ad1own[:, 1 + j0:1 + j0 + JB, :],
                                   in_=psad[:])
                dall1 = pat1.tile([128, 2, NW, 8], f16, tag="dall")
                build_dall(ad1own, 8, dall1, ppd1)
                at_fill(dall1, at1_all, 8, pat1, ppat1)

                # replicated table transform (emitted after at1 so the DVE
                # queue holds only at1 work; evictions go to Act only)
                AB = 10                      # chunks per DMA batch
                for cb in range(8):
                    base = cb * NLOC
                    for jj in range(0, 100, AB):
                        nb = min(AB, 100 - jj)
                        xt = pa.tile([128, 2, AB * 128], f16, tag="xt")
                        for k in range(2):
                            nc.sync.dma_start(
                                xt[:, k, 0:nb * 128],
                                xTfull[k, :, base + jj * 128:
                                       base + (jj + nb) * 128])
                        row = pa.tile([128, AB, 128], f16, tag="row")
                        for u in range(0, nb, 4):
                            ub = min(4, nb - u)
                            ps = ppa.tile([128, 4, 80], f32, tag="np1")
                            for j in range(ub):
                                for k in range(2):
                                    nc.tensor.matmul(
                                        ps[:, j, :],
                                        lhsT=xt[:, k, (u + j) * 128:
                                                (u + j + 1) * 128],
                                        rhs=w1_sb[:, k, :], start=(k == 0),
                                        stop=(k == 1))
                            nc.scalar.copy(out=row[:, u:u + ub, 0:80],
                                           in_=ps[:, 0:ub, :])
                        nc.sync.dma_start(
                            bass.AP(tab1q[cb // 2],
                                    ((cb % 2) * NLOC + jj) * 128,
                                    [[NW * 128, 128], [256, nb // 2],
                                     [1, 256]]),
                            row[:, 0:nb, :])



            # ---------- phase B: layer-1 edge pass ----------
            def edge_pass(layer, pb_ext=None, ph_ext=None):
                if layer == 1:
                    ncols_h, as_col = 72, 64
                    nheads, msgw = 8, 72
                else:
                    ncols_h, as_col = 41, 40
                    nheads, msgw = 1, 41
                pool_name = f"pe{layer}"
                import contextlib
                pbc = (contextlib.nullcontext(pb_ext) if pb_ext is not None
                       else tc.tile_pool(name=pool_name, bufs=2))
                phc = (contextlib.nullcontext(ph_ext) if ph_ext is not None
                       else tc.tile_pool(name=pool_name + "h", bufs=2))
                with (pbc as pb,
                      phc as ph,
                      tc.tile_pool(name=pool_name + "m", bufs=1) as pm,
                      tc.tile_pool(name=pool_name + "w",
                                   bufs=(2 if layer == 1 else 3)) as pw,
                      tc.tile_pool(name=pool_name + "p", bufs=2,
                                   space="PSUM") as ppb):
                    qn = 0

                    def gather_sq(s, q, hix, ht):
                        nonlocal qn
                        k0, _ = sb_chunks[s]
                        for (ss, qq, slot0, nids) in plan["hcalls"]:
                            if ss != s or qq != q:
                                continue
                            c0 = slot0 // 128 - k0
                            if layer == 1:
                                tsl = tab1q[q][:, 0:ncols_h]
                            else:
                                tsl = ago[q * QS:(q + 1) * QS, 0:ncols_h]
                            _gather_small(
                                nc.gpsimd,
                                ht[:, c0:c0 + nids // 128, :],
                                tsl,
                                hix[:, (slot0 - k0 * 128) // 16:
                                    (slot0 - k0 * 128 + nids) // 16],
                                nids, ncols_h, 128, queue_num=qn % 4)
                            qn += 1

                    pre = l1pre if layer == 1 else {}
                    for s in range(NSB):
                        k0, k1 = sb_chunks[s]
                        cps = k1 - k0
                        if s in pre:
                            hix, ht = pre[s]
                        else:
                            hix = pb.tile([128, CPSB_MAX * 8], i16,
                                          tag="hix")
                            nc.sync.dma_start(hix[:, 0:cps * 8],
                                              hidx_d[:, k0 * 8:k1 * 8])
                            ht = ph.tile([128, CPSB_MAX, ncols_h], f16,
                                         tag="ht")
                            for q in range(NQ):
                                gather_sq(s, q, hix, ht)
                        if layer == 1:
                            at_s = at1_all[:, k0:k1, :]
                        else:
                            at_s = at2_all[:, k0:k1, 0:1]

                        e = pm.tile([128, CPSB_MAX, nheads], f32, tag="e")
                        lr = e
                        nc.vector.tensor_tensor(
                            out=e[:, 0:cps, :],
                            in0=ht[:, 0:cps, as_col:as_col + nheads],
                            in1=at_s, op=OP.add)
                        nc.vector.scalar_tensor_tensor(
                            out=lr[:, 0:cps, :], in0=e[:, 0:cps, :],
                            scalar=0.2, in1=e[:, 0:cps, :],
                            op0=OP.mult, op1=OP.max)

                        msg = pm.tile([128, CPSB_MAX, msgw], bf16, tag="msg")
                        # w into msg's trailing cols (compact exp)
                        nc.scalar.activation(
                            out=msg[:, 0:cps, as_col:as_col + nheads],
                            in_=lr[:, 0:cps, :], func=AF.Exp)
                        if layer == 1:
                            # expanded weights for a clean 2x-mode mult
                            half = (CPSB_MAX + 1) // 2
                            wgx = pm.tile([128, half, 8, 8], bf16, tag="wgx")
                            for h0 in (0, half):
                                hn = min(half, cps - h0)
                                if hn <= 0:
                                    continue
                                lrs = lr[:, h0:h0 + hn, :]
                                nc.scalar.activation(
                                    out=wgx[:, 0:hn, :, :],
                                    in_=BC(lrs, [lrs.ap[0], lrs.ap[1],
                                                 lrs.ap[2], [0, 8]]),
                                    func=AF.Exp)
                                m_ = msg[:, h0:h0 + hn, 0:64]
                                h_ = ht[:, h0:h0 + hn, 0:64]
                                nc.vector.tensor_tensor(
                                    out=BC(m_, [m_.ap[0], m_.ap[1],
                                                [8, 8], [1, 8]]),
                                    in0=BC(h_, [h_.ap[0], h_.ap[1],
                                                [8, 8], [1, 8]]),
                                    in1=wgx[:, 0:hn, :, :], op=OP.mult)
                        else:
                            wgx2 = pw.tile([128, CPSB_MAX, 40], bf16,
                                           tag="wgx2")
                            lrs = lr[:, 0:cps, :]
                            nc.scalar.activation(
                                out=wgx2[:, 0:cps, :],
                                in_=BC(lrs, [lrs.ap[0], lrs.ap[1], [0, 40]]),
                                func=AF.Exp)
                            nc.vector.tensor_tensor(
                                out=msg[:, 0:cps, 0:40],
                                in0=ht[:, 0:cps, 0:40],
                                in1=wgx2[:, 0:cps, :], op=OP.mult)

                        # windows: one-hot + aggregation matmuls, PSUM
                        # evicted into a per-sb batch tile
                        hsb = pm.tile([128, NWSB, msgw], f32, tag="hsb")
                        for wi in range(NWSB):
                            w = s * NWSB + wi
                            cols = wcols[w]
                            cpw = len(cols)
                            c0 = colbase[w]
                            ohT = pw.tile([128, 128, MAXCPW], bf16, tag="ohT")
                            dr = drel_sb[:, c0:c0 + cpw]
                            nc.vector.tensor_tensor(
                                out=ohT[:, :, 0:cpw],
                                in0=BC(dr, [dr.ap[0], [0, 128], dr.ap[1]]),
                                in1=iota_rep[:, :, 0:cpw], op=OP.is_equal)
                            ps = ppb.tile([128, msgw], f32, tag="agg")
                            for i, k in enumerate(cols):
                                nc.tensor.matmul(
                                    ps[:], lhsT=ohT[:, :, i],
                                    rhs=msg[:, k - k0, :],
                                    start=(i == 0), stop=(i == cpw - 1))
                            nc.scalar.copy(out=hsb[:, wi, :], in_=ps[:])

                        # per-sb batched softmax-normalize (+ elu/r2 for L1)
                        if layer == 1:
                            den = pw.tile([128, NWSB, 8], f32, tag="den")
                            nc.scalar.activation(out=den[:],
                                                 in_=hsb[:, :, 64:72],
                                                 func=AF.Copy, bias=ACC_EPS)
                            rec = pw.tile([128, NWSB, 8], f32, tag="rec")
                            nc.vector.reciprocal(
                                rec[:].rearrange("p a b -> p (a b)"),
                                den[:].rearrange("p a b -> p (a b)"))
                            o1 = pw.tile([128, NWSB, 64], f32, tag="o1")
                            nu = hsb[:, :, 0:64]
                            r_ = rec[:]
                            nc.vector.tensor_tensor(
                                out=BC(o1[:], [o1[:].ap[0], [64, NWSB],
                                               [8, 8], [1, 8]]),
                                in0=BC(nu, [nu.ap[0], [72, NWSB],
                                            [8, 8], [1, 8]]),
                                in1=BC(r_, [r_.ap[0], [8, NWSB],
                                            [1, 8], [0, 8]]),
                                op=OP.mult)
                            o1v = o1[:].rearrange("p a b -> p (a b)")
                            if not plan["skip_b1"]:
                                b1w = b1r[:]
                                nc.vector.tensor_tensor(
                                    out=o1v,
                                    in0=o1v,
                                    in1=BC(b1w, [b1w.ap[0], [0, NWSB],
                                                 [1, 64]]),
                                    op=OP.add)
                            # elu = relu(x) + exp(-relu(-x)) - 1
                            rneg = pw.tile([128, NWSB, 64], f32, tag="rneg")
                            nc.scalar.activation(
                                out=rneg[:].rearrange("p a b -> p (a b)"),
                                in_=o1v, func=AF.Relu, scale=-1.0)
                            expn = rneg
                            nc.scalar.activation(
                                out=expn[:].rearrange("p a b -> p (a b)"),
                                in_=rneg[:].rearrange("p a b -> p (a b)"),
                                func=AF.Exp, scale=-1.0)
                            pos = pw.tile([128, NWSB, 64], f32, tag="pos")
                            nc.scalar.activation(
                                out=pos[:].rearrange("p a b -> p (a b)"),
                                in_=o1v, func=AF.Relu)
                            hl16 = pw.tile([128, NWSB, 64], f16, tag="hl16")
                            nc.vector.scalar_tensor_tensor(
                                out=hl16[:].rearrange("p a b -> p (a b)"),
                                in0=expn[:].rearrange("p a b -> p (a b)"),
                                scalar=-1.0,
                                in1=pos[:].rearrange("p a b -> p (a b)"),
                                op0=OP.add, op1=OP.add)
                            r2s = pw.tile([128, NWSB, 42], f16, tag="r2s")
                            for wi in range(NWSB):
                                pst = ppb.tile([64, 128], f16, tag="tr")
                                nc.tensor.transpose(out=pst[:],
                                                    in_=hl16[:, wi, :],
                                                    identity=ident[:])
                                hlT = pw.tile([64, 128], f16, tag="hlT")
                                nc.scalar.copy(out=hlT[:], in_=pst[:])
                                r2p = ppb.tile([128, 42], f32, tag="r2p")
                                nc.tensor.matmul(r2p[:], lhsT=hlT[:],
                                                 rhs=w2_sb[:], start=True,
                                                 stop=True)
                                nc.scalar.copy(out=r2s[:, wi, :], in_=r2p[:])
                            nc.sync.dma_start(
                                bass.AP(agi2, (s * NWSB) * 128,
                                        [[NW * 128, 128], [128, NWSB],
                                         [1, 42]]),
                                r2s[:])
                            # stash a_dst2 (col 41) for the L2 telescoping
                            nc.scalar.copy(
                                out=ad2own[:, 1 + s * NWSB:
                                           1 + (s + 1) * NWSB, :],
                                in_=r2s[:, :, 41:42])
                        else:
                            den = pw.tile([128, NWSB, 1], f32, tag="den2")
                            nc.scalar.activation(out=den[:],
                                                 in_=hsb[:, :, 40:41],
                                                 func=AF.Copy, bias=ACC_EPS)
                            rec = pw.tile([128, NWSB, 1], f32, tag="rec2")
                            nc.vector.reciprocal(
                                rec[:].rearrange("p a b -> p (a b)"),
                                den[:].rearrange("p a b -> p (a b)"))
                            o2 = pw.tile([128, NWSB, 40], f32, tag="o2")
                            nu = hsb[:, :, 0:40]
                            r_ = rec[:]
                            nc.vector.tensor_tensor(
                                out=o2[:],
                                in0=BC(nu, [nu.ap[0], [41, NWSB], [1, 40]]),
                                in1=BC(r_, [r_.ap[0], [1, NWSB], [0, 40]]),
                                op=OP.mult)
                            o2v = o2[:].rearrange("p a b -> p (a b)")
                            if not plan["skip_b2"]:
                                b2w = b2r[:]
                                nc.vector.tensor_tensor(
                                    out=o2v, in0=o2v,
                                    in1=BC(b2w, [b2w.ap[0], [0, NWSB],
                                                 [1, 40]]),
                                    op=OP.add)
                            nc.sync.dma_start(
                                bass.AP(out, (s * NWSB) * 128 * 40,
                                        [[40, 128], [128 * 40, NWSB],
                                         [1, 40]]),
                                o2[:])

            edge_pass(1, pb_ext=pb1, ph_ext=ph1)
            estack.close()

            # ---------- AllGather first; the at2 telescoping (local data
            # only) runs during the collective ----------
            nc.gpsimd.collective_compute(
                "AllGather", OP.bypass, ins=[agi2[:]], outs=[ago[:]],
                replica_groups=[list(range(8))])

            with (tc.tile_pool(name="pat2", bufs=2) as pat2,
                  tc.tile_pool(name="ppat2", bufs=2, space="PSUM") as ppat2,
                  tc.tile_pool(name="ppd2", bufs=1, space="PSUM") as ppd2):
                dall2 = pat2.tile([128, 2, NW, 1], f16, tag="dall")
                build_dall(ad2own, 1, dall2, ppd2)
                at_fill(dall2, at2_all, 1, pat2, ppat2)

            edge_pass(2)

    nc.finalize()
    return nc


def kernel(**inputs):
    per_core, plan = _host_prep(**inputs)
    if "nc" not in _CACHE:
        _CACHE["nc"] = _build_nc(plan)
    nc = _CACHE["nc"]
    from concourse.bass_utils import run_bass_kernel_spmd
    res = run_bass_kernel_spmd(nc, per_core, list(range(8)))
    full = np.concatenate([res.results[c]["out"] for c in range(8)], axis=0)
    return np.ascontiguousarray(full[:N]).astype(np.float32)



# revision 80
# speedup vs baseline: 1.0606x; 1.0144x over previous
"""GAT (2-layer PyG GATConv, eval) on 8 Trainium2 NeuronCores.

Sharding: nodes range-partitioned (NLOC=12800/core); core c owns edges whose
dst is in its range. The layer-1 node table is REPLICATED: every core
transforms all 102400 rows from the full x (the x stream + table write cost
less on the DMA timeline than the AllGather they replace, and the a_dst
telescoping runs concurrently on DVE from the core's own x shard). Layer 2's
table still needs one AllGather (agi2 -> ago), overlapped by the at2
telescoping machinery.

Slot layout per core: superblock (10 windows) -> quadrant -> window, with
per-(window,quadrant) STATIC capacities = max edge count over the 8 cores
(SPMD: one module runs on all cores; only tensor contents differ) — ~10%
slot padding vs 28% for fixed-size groups. Gather calls use
single_packet=False, which lets the SWDGE ucode stream descriptors through
the ring: up to 4x the ring (dynamic_dma_scratch_size/16) indices per call
(probed on HW: 8192 idx streams through a 2048-desc ring; 8x crashes;
single-packet calls hard-crash above 1024 idx). One ~5.5k-idx call per
(sb,quadrant) segment cuts the 994ns-per-call Pool overhead ~5x vs the
1024-idx baseline.

Per layer, per edge slot: a 144B/82B payload gather pulls [h|a_src] rows
(256B-stride tables, int16 idx into 25600-row quadrants). a_dst is NOT
gathered: slots are drel-sorted inside each (window,quadrant) group, so
per-slot a_dst[drel] telescopes — P[d,s] = (s >= start[d]) is a DVE is_ge
step matrix (host-static start tables) and PE computes P^T @ diff(a_dst)
per chunk column; window starts mid-chunk use a compensated diff table
(row0 = ad[0,w]-ad[127,w-1]) accumulated in the same PSUM. Both at1 and at2
expansions run during their layer's AllGather (they read only local data).
Softmax is the shift-invariant no-max form (w = exp(leakyrelu(as+ad)),
|e| < ~25 so fp32 exp is safe). Segment reduction is a PE matmul whose
stationary matrix is a transposed one-hot built by DVE is_equal in the
2x-mode layout; boundary chunks carry a masked drel column per touching
window. msg = h*w uses an Act-expanded weight tile so the DVE mult runs in
2x mode. Softmax normalize + ELU + the r2 = hlT @ [W2|a2_src|a2_dst]
projection are batched per superblock; tables are written node-permuted
(row = p*100 + j within each core block) so writes coalesce per partition.
"""
import numpy as np
import ml_dtypes

N = 100000
E = 1600000
NF = 256
HEADS, NHID = 8, 8
NH = HEADS * NHID          # 64
NCLASS = 40
NLOC = 12800               # nodes per core
NW = 100                   # 128-dst windows per core
NQ = 4                     # src table quadrants
QS = 25600                 # rows per quadrant
NWSB = 10                  # windows per superblock
NSB = NW // NWSB           # 10 superblocks
NTOT = 102400
ACC_EPS = 1e-16

_CACHE = {}


def _ceil128(x):
    return (x + 127) & ~127


def _host_prep(x, edge_index, W1, a1_src, a1_dst, b1, W2, a2_src, a2_dst, b2):
    src = np.asarray(edge_index[0], dtype=np.int64)
    dst = np.asarray(edge_index[1], dtype=np.int64)

    # table-row permutation: node n -> row  c*NLOC + (l%128)*NW + l//128
    def rowperm(n):
        c = n // NLOC
        l = n - c * NLOC
        return c * NLOC + (l % 128) * NW + l // 128

    srow = rowperm(src)
    sq = srow // QS
    sidx = (srow - sq * QS).astype(np.int16)

    core = dst // NLOC
    dloc = dst - core * NLOC
    w_e = (dloc >> 7).astype(np.int64)
    dr_e = (dloc & 127).astype(np.int64)
    adidx = (dr_e * NW + w_e).astype(np.int16)
    sb_e = w_e // NWSB

    # static capacities: max over cores per (window, quadrant); >=1 so every
    # group is present in the slot stream (the telescoped a_dst expansion
    # needs window w-1 to precede window w inside each (sb,q) segment)
    gkey = (core * NW + w_e) * NQ + sq          # [E]
    cnt = np.bincount(gkey, minlength=8 * NW * NQ).reshape(8, NW, NQ)
    cap = np.maximum(cnt.max(axis=0), 1)         # [NW, NQ]

    # slot layout: sb -> quadrant -> window.  With single_packet=False the
    # SWDGE gather ucode streams descriptors through the ring, so calls up
    # to 4x the ring size (dynamic_dma_scratch_size/16) are fine (probed on
    # HW: 8192 idx streams through a 2048-desc ring; 8x crashes).
    MAXIDX = 8192
    wq_start = np.zeros((NW, NQ), np.int64)
    hcalls = []                                  # (sb, q, slot0, n_idx)
    sb_chunks = []                               # (k0, k1) per sb
    nslot = 0
    for s in range(NSB):
        k0 = nslot // 128
        for q in range(NQ):
            seg0 = nslot
            for w in range(s * NWSB, (s + 1) * NWSB):
                wq_start[w, q] = nslot
                nslot += int(cap[w, q])
            nslot = _ceil128(nslot)
            for off in range(seg0, nslot, MAXIDX):
                hcalls.append((s, q, off, min(MAXIDX, nslot - off)))
        sb_chunks.append((k0, nslot // 128))
    NSLOT = nslot
    NCHUNK = NSLOT // 128

    # window label per slot (shared across cores: layout is static).  Group
    # pads inherit their group's window; (sb,q)-tail ceil128 pads inherit the
    # segment's last window.
    wfull = np.zeros(NSLOT, np.int64)
    for s in range(NSB):
        for q in range(NQ):
            for w in range(s * NWSB, (s + 1) * NWSB):
                a = int(wq_start[w, q])
                e_ = a + int(cap[w, q])
                wfull[a:e_] = w
            wfull[e_:_ceil128(e_)] = (s + 1) * NWSB - 1   # segment tail pads

    # at-plan: per chunk, one column-copy per touching window; copies of one
    # chunk accumulate in PSUM.  v=1 (compensated diff table) iff the window
    # starts mid-chunk.  Packed into segments of <= ATSEGC columns.
    ATSEGC = 24
    atsegs = []            # (kg0, nk, c0, cols=[(i, klocal, v, w, first, last)])
    copies_per_chunk = []  # [(w, a, b)]
    for k in range(NCHUNK):
        wk = wfull[k * 128:(k + 1) * 128]
        runs = []
        a = 0
        for p in range(1, 128):
            if wk[p] != wk[p - 1]:
                runs.append((int(wk[a]), a, p))
                a = p
        runs.append((int(wk[a]), a, 128))
        copies_per_chunk.append(runs)
    ncolat = 0
    cur = None
    colat_of = {}
    for k in range(NCHUNK):
        runs = copies_per_chunk[k]
        if cur is None or cur[3] + len(runs) > ATSEGC:
            if cur is not None:
                atsegs.append(cur)
            cur = [k, 0, ncolat, 0, []]
        kloc = cur[1]
        for i, (w, a, b) in enumerate(runs):
            colat_of[(k, a)] = ncolat
            cur[4].append((cur[3], kloc, 1 if a > 0 else 0, w,
                           i == 0, i == len(runs) - 1))
            cur[3] += 1
            ncolat += 1
        cur[1] += 1
    if cur is not None:
        atsegs.append(cur)
    NCOLAT = ncolat

    # per-window chunk columns
    colmap = np.full((NW, NCHUNK), -1, np.int64)
    wcols = []                                   # per w: (colbase, [chunks])
    ncol = 0
    for w in range(NW):
        cols = []
        for q in range(NQ):
            a = int(wq_start[w, q])
            b = a + int(cap[w, q])
            for k in range(a // 128, (b + 127) // 128):
                cols.append(k)
                colmap[w, k] = ncol
                ncol += 1
        wcols.append(cols)
    NCOL = ncol
    MAXCPW = max(len(c) for c in wcols)

    plan = {
        "NSLOT": NSLOT, "NCHUNK": NCHUNK, "NCOL": NCOL, "MAXCPW": MAXCPW,
        "NCOLAT": NCOLAT, "ATSEGC": ATSEGC,
        "hcalls": hcalls, "atsegs": atsegs, "sb_chunks": sb_chunks,
        "wcols": wcols,
        "skip_b1": bool(np.all(np.asarray(b1) == 0)),
        "skip_b2": bool(np.all(np.asarray(b2) == 0)),
    }

    # group-id in slot order: (sb, q, w_in_sb)
    flatg = (sb_e * NQ + sq) * NWSB + (w_e - sb_e * NWSB)
    gstart_flat = np.zeros(NSB * NQ * NWSB, np.int64)
    for s in range(NSB):
        for q in range(NQ):
            for wi in range(NWSB):
                gstart_flat[(s * NQ + q) * NWSB + wi] = wq_start[s * NWSB + wi, q]

    per_core = []
    hidx_all, startd_all, drel_all = [], [], []
    for c in range(8):
        m = core == c
        fg = flatg[m]
        drc = dr_e[m]
        order = np.lexsort((drc, fg))
        fgs = fg[order]
        cntc = np.bincount(fgs, minlength=NSB * NQ * NWSB)
        starts = np.zeros_like(cntc)
        starts[1:] = np.cumsum(cntc)[:-1]
        rank = np.arange(len(fgs)) - starts[fgs]
        slot = gstart_flat[fgs] + rank

        hvec = np.zeros(NSLOT, np.int16)
        hvec[slot] = sidx[m][order]

        drel = np.full((128, NCOL), 128.0, np.float32)
        k_s = slot >> 7
        p_s = slot & 127
        we_s = w_e[m][order]
        col_s = colmap[we_s, k_s]
        assert (col_s >= 0).all()
        drel[p_s, col_s] = drc[order].astype(np.float32)

        # per-slot drel stream (pads = 128) for the telescoped start tables
        drfull = np.full(NSLOT, 128, np.int64)
        drfull[slot] = drc[order]
        startd = np.zeros((128, NCOLAT), np.float32)
        dgrid = np.arange(128)
        for k in range(NCHUNK):
            for (w, a, b) in copies_per_chunk[k]:
                col = colat_of[(k, a)]
                drs = drfull[k * 128 + a:k * 128 + b]
                startd[:, col] = a + np.searchsorted(drs, dgrid)

        def wrap16(v):
            o = np.zeros((128, NSLOT // 16), np.int16)
            sl = np.arange(NSLOT)
            o[sl % 16, sl // 16] = v
            for r in range(1, 8):
                o[16 * r:16 * (r + 1)] = o[:16]
            return o

        hidx_all.append(wrap16(hvec))
        startd_all.append(startd.astype(np.float16))
        drel_all.append(drel.astype(ml_dtypes.bfloat16))

    # weights
    W1 = np.asarray(W1, np.float32)
    v_s1 = np.einsum("chk,hk->ch", W1.reshape(NF, HEADS, NHID),
                     np.asarray(a1_src, np.float32))
    v_d1 = np.einsum("chk,hk->ch", W1.reshape(NF, HEADS, NHID),
                     np.asarray(a1_dst, np.float32))
    W1e = np.concatenate([W1, v_s1, v_d1], axis=1).reshape(2, 128, 80)
    W1e = W1e.astype(np.float16)

    W2 = np.asarray(W2, np.float32)
    v_s2 = W2 @ np.asarray(a2_src, np.float32)[0]
    v_d2 = W2 @ np.asarray(a2_dst, np.float32)[0]
    W2e = np.concatenate([W2, v_s2[:, None], v_d2[:, None]],
                         axis=1).astype(np.float16)   # [64, 42]

    xp = np.zeros((NTOT, NF), np.float32)
    xp[:N] = np.asarray(x, np.float32)

    # lhsT matrices for building the diff tables on PE:
    # mshift[:,0,:] = Mplain^T (fwd diff), mshift[:,1,:] = -sel(127)->row0
    mshiftT = np.zeros((128, 2, 128), np.float16)
    mshiftT[:, 0, :] = (np.eye(128) - np.eye(128, k=1)).astype(np.float16)
    mshiftT[127, 1, 0] = -1.0

    # full transposed x, shared by all cores: the layer-1 table transform is
    # replicated (each core computes all 102400 rows locally; the x stream +
    # table write fit under what the AllGather used to cost, and the a_dst
    # telescoping runs concurrently from the core's own x shard)
    xT = np.ascontiguousarray(xp.T).astype(np.float16).reshape(2, 128, NTOT)

    for c in range(8):
        xloc = np.ascontiguousarray(xp[c * NLOC:(c + 1) * NLOC].T)
        per_core.append({
            "xTloc": xloc.astype(np.float16).reshape(2, 128, NLOC),
            "xTfull": xT,
            "W1e": W1e,
            "W2e": W2e,
            "b1": np.asarray(b1, np.float32)[None, :],
            "b2": np.asarray(b2, np.float32)[None, :],
            "hidx": hidx_all[c],
            "startd": startd_all[c],
            "drel": drel_all[c],
            "mshift": mshiftT,
        })
    return per_core, plan


def _gather_small(g, out_ap, in_ap, idxs_ap, num_idxs, elem_size, elem_step,
                  queue_num=0, single_packet=False):
    """dma_gather with payload < 256B; only the 256B row-stride rule is real
    for the non-transpose path."""
    import concourse.mybir as mybir
    stride_bytes = elem_step * mybir.dt.size(in_ap.dtype)
    assert stride_bytes % 256 == 0
    _in_ap = g.lower_ap_dma(in_ap, for_custom_bir_dma=True)
    _idxs_ap = g.lower_ap(idxs_ap)
    _out_ap = g.lower_ap(out_ap)
    return g.add_instruction(mybir.InstDMAGatherAnt(
        name=g.bass.get_next_instruction_name(),
        ins=[*_in_ap, _idxs_ap, g.lower_val_access(g.to_reg(num_idxs))],
        outs=[_out_ap],
        transpose=False,
        num_idxs=num_idxs,
        elem_size=elem_size,
        stride_bytes_256=stride_bytes // 256,
        gen_mode=0,
        single_packet=single_packet,
        queue_num=queue_num,
        sbuf_tokens_per_rank=0,
        sbuf_free_dim_per_rank=0,
        sbuf_free_dim_pad_per_rank=0,
        sbuf_byte_offset=0,
    ))


def _build_nc(plan):
    import concourse.bass as bass
    import concourse.bacc as bacc
    import concourse.mybir as mybir
    import concourse.tile as tile
    from concourse.library_config import mlp
    from concourse.masks import make_identity

    f32, f16, bf16, i16 = (mybir.dt.float32, mybir.dt.float16,
                           mybir.dt.bfloat16, mybir.dt.int16)
    AF = mybir.ActivationFunctionType
    OP = mybir.AluOpType

    NSLOT = plan["NSLOT"]
    NCOL = plan["NCOL"]
    MAXCPW = plan["MAXCPW"]
    sb_chunks = plan["sb_chunks"]
    wcols = plan["wcols"]
    CPSB_MAX = max(k1 - k0 for k0, k1 in sb_chunks)
    colbase = [0] * NW
    for w in range(1, NW):
        colbase[w] = colbase[w - 1] + len(wcols[w - 1])

    NCOLAT = plan["NCOLAT"]
    ATSEGC = plan["ATSEGC"]
    atsegs = plan["atsegs"]

    nc = bacc.Bacc("TRN2", target_bir_lowering=False, debug=False,
                   num_devices=8, num_swdge_queues=4,
                   dynamic_dma_scratch_size=32768)

    xTloc = nc.dram_tensor("xTloc", [2, 128, NLOC], f16, kind="ExternalInput")
    xTfull = nc.dram_tensor("xTfull", [2, 128, NTOT], f16,
                            kind="ExternalInput")
    W1e = nc.dram_tensor("W1e", [2, 128, 80], f16, kind="ExternalInput")
    W2e = nc.dram_tensor("W2e", [64, 42], f16, kind="ExternalInput")
    b1 = nc.dram_tensor("b1", [1, 64], f32, kind="ExternalInput")
    b2 = nc.dram_tensor("b2", [1, 40], f32, kind="ExternalInput")
    hidx_d = nc.dram_tensor("hidx", [128, NSLOT // 16], i16,
                            kind="ExternalInput")
    startd_d = nc.dram_tensor("startd", [128, NCOLAT], f16,
                              kind="ExternalInput")
    mshift_d = nc.dram_tensor("mshift", [128, 2, 128], f16,
                              kind="ExternalInput")
    drel_d = nc.dram_tensor("drel", [128, NCOL], bf16, kind="ExternalInput")
    out = nc.dram_tensor("out", [NLOC, 40], f32, kind="ExternalOutput")

    # per-QUADRANT table tensors: (sb,q) gathers depend only on quadrant q's
    # transform writes, so the gather stream overlaps the transform tail
    tab1q = [nc.dram_tensor(f"tab1q{q}", [QS, 128], f16) for q in range(4)]
    agi2 = nc.dram_tensor("agi2", [NLOC, 128], f16)    # [h2|as2|ad2|pad]
    ago = nc.dram_tensor("ago", [NTOT, 128], f16, addr_space="Shared")

    def BC(ap, dims):
        return bass.AP(ap.tensor, ap.offset, dims)

    def dram_rows(t, offset_rows, dims):
        """AP into DRAM tensor t (row-major, 128 f16 cols) at row offset."""
        return bass.AP(t, offset_rows * 128, dims)

    with tile.TileContext(nc) as tc:
        with tc.tile_pool(name="const", bufs=1) as pc:
            nc.gpsimd.load_library(mlp)

            drel_sb = pc.tile([128, NCOL], bf16)
            nc.sync.dma_start(drel_sb[:], drel_d[:])
            w1_sb = pc.tile([128, 2, 80], f16)
            nc.sync.dma_start(w1_sb[:], W1e[:].rearrange("k p n -> p k n"))
            w2_sb = pc.tile([64, 42], f16)
            nc.sync.dma_start(w2_sb[:], W2e[:])

            NIOTA = max(MAXCPW, ATSEGC)
            ii = pc.tile([128, 128, NIOTA], i16)
            nc.gpsimd.iota(ii[:], pattern=[[1, 128], [0, NIOTA]], base=0,
                           channel_multiplier=0)
            iota_rep = pc.tile([128, 128, MAXCPW], bf16)
            nc.vector.tensor_copy(out=iota_rep[:], in_=ii[:, :, 0:MAXCPW])
            iota_at = pc.tile([128, 128, ATSEGC], f16)
            nc.vector.tensor_copy(out=iota_at[:], in_=ii[:, :, 0:ATSEGC])
            startd_sb = pc.tile([128, NCOLAT], f16)
            nc.sync.dma_start(startd_sb[:], startd_d[:])
            mshift_sb = pc.tile([128, 2, 128], f16)
            nc.sync.dma_start(mshift_sb[:], mshift_d[:])

            ident = pc.tile([128, 128], f16)
            make_identity(nc, ident[:])

            ones32 = pc.tile([1, 128], f32)
            nc.vector.memset(ones32[:], 1.0)

            b1r = pc.tile([128, 64], f32)
            b2r = pc.tile([128, 40], f32)
            with tc.tile_pool(name="pini", bufs=2, space="PSUM") as ppi:
                for row_d, width, dest in ((b1, 64, b1r), (b2, 40, b2r)):
                    t = pc.tile([1, width], f32, tag=f"rrow{width}")
                    nc.sync.dma_start(t[:], row_d[:])
                    ps = ppi.tile([128, width], f32, tag="rep")
                    nc.tensor.matmul(ps[:], lhsT=ones32[:], rhs=t[:],
                                     start=True, stop=True)
                    nc.vector.tensor_copy(out=dest[:], in_=ps[:])

            # a_dst per-slot tiles, telescoped from the own-block a_dst
            # values via PE (P[d,s] = (s >= start[d]) is a step matrix;
            # P @ diff(ad) = ad[drel[s]] since slots are drel-sorted)
            at1_all = pc.tile([128, plan["NCHUNK"], 8], f16)
            at2_all = pc.tile([128, plan["NCHUNK"], 1], f16)
            ad2own = pc.tile([128, NW + 1, 1], f16)
            nc.vector.memset(ad2own[:, 0:1, :].rearrange("p a b -> p (a b)"),
                             0.0)

            def at_fill(dall, at_tile, H, pat, ppat):
                for (kg0, nk, c0, ncols, cols) in atsegs:
                    P = pat.tile([128, 128, ATSEGC], f16, tag="P")
                    st_ = startd_sb[:, c0:c0 + ncols]
                    nc.vector.tensor_tensor(
                        out=P[:, :, 0:ncols],
                        in0=iota_at[:, :, 0:ncols],
                        in1=BC(st_, [st_.ap[0], [0, 128], st_.ap[1]]),
                        op=OP.is_ge)
                    ps = ppat.tile([128, ATSEGC, 8], f32, tag="atps")
                    for (i, klocal, v, w, first, last) in cols:
                        nc.tensor.matmul(ps[:, klocal, 0:H],
                                         lhsT=P[:, :, i],
                                         rhs=dall[:, v, w, 0:H],
                                         start=first, stop=last)
                    nc.scalar.activation(out=at_tile[:, kg0:kg0 + nk, :],
                                         in_=ps[:, 0:nk, 0:H], func=AF.Copy)

            def build_dall(adown, H, dall, ppd):
                # adown: [128, NW+1, H] f16, col 0 zeroed.
                # dall[:,0,w,:] = fwd-diff (row d: ad[d]-ad[d-1], row0 ad[0]),
                # dall[:,1,w,:] = same but row0 = ad[0,w]-ad[127,w-1].
                # PSUM rows padded to whole banks (512 f32); each matmul's
                # output chunk must stay inside one bank.  The two diff
                # variants run sequentially through one 2-bank tile.
                psrow = ((NW * H + 511) // 512) * 512
                a_ = adown[:]
                for v in range(2):
                    ps = ppd.tile([128, psrow], f32, tag=f"dps{H}")
                    for off in range(0, NW * H, 512):
                        nn = min(512, NW * H - off)
                        rhs_cur = bass.AP(a_.tensor, a_.offset + H + off,
                                          [a_.ap[0], [1, nn]])
                        rhs_prev = bass.AP(a_.tensor, a_.offset + off,
                                           [a_.ap[0], [1, nn]])
                        nc.tensor.matmul(ps[:, off:off + nn],
                                         lhsT=mshift_sb[:, 0, :],
                                         rhs=rhs_cur, start=True,
                                         stop=(v == 0))
                        if v == 1:
                            nc.tensor.matmul(ps[:, off:off + nn],
                                             lhsT=mshift_sb[:, 1, :],
                                             rhs=rhs_prev, start=False,
                                             stop=True)
                    nc.scalar.activation(
                        out=dall[:, v, :, :].rearrange("p b c -> p (b c)"),
                        in_=ps[:, 0:NW * H], func=AF.Copy)

            # Allocate the L1 gather pools BEFORE (under) the transform pools
            # in the stack allocator: address-disjoint tiles carry no WAR
            # dep on the transform scope, and with per-quadrant table
            # tensors the (sb,q) gathers start as soon as quadrant q lands.
            import contextlib
            estack = contextlib.ExitStack()
            pb1 = estack.enter_context(tc.tile_pool(name="pe1", bufs=2))
            ph1 = estack.enter_context(tc.tile_pool(name="pe1h", bufs=2))

            l1pre = {}

            # ---------- phase A (REPLICATED, no collective): the full-table
            # transform streams xTfull on DMA/PE/Act while, concurrently,
            # DVE runs the at1 telescoping.  ad1own = x_own @ v_d1 is
            # computed directly from the core's own x shard (same fused math
            # as the table's a_dst columns), so at1 never touches tab1;
            # engines are disjoint: DMA+PE+Act = transform, DVE+PE = at1.
            with (tc.tile_pool(name="pa", bufs=3) as pa,
                  tc.tile_pool(name="ppa", bufs=2, space="PSUM") as ppa,
                  tc.tile_pool(name="pat1", bufs=2) as pat1,
                  tc.tile_pool(name="ppat1", bufs=2, space="PSUM") as ppat1,
                  tc.tile_pool(name="ppd1", bufs=1, space="PSUM") as ppd1):
                # ad1own: per j-block matmul x_own[:, j] @ v_d1 (cols 72:80
                # of W1e), accumulated over the two k-halves
                ad1own = pat1.tile([128, NW + 1, 8], f16, tag="adown")
                nc.vector.memset(
                    ad1own[:, 0:1, :].rearrange("p a b -> p (a b)"), 0.0)
                JB = 10
                for j0 in range(0, NW, JB):
                    xo = pat1.tile([128, 2, JB * 128], f16, tag="xo")
                    for k in range(2):
                        nc.sync.dma_start(
                            xo[:, k, :],
                            xTloc[k, :, j0 * 128:(j0 + JB) * 128])
                    psad = ppat1.tile([128, JB, 8], f32, tag="psad")
                    for j in range(JB):
                        for k in range(2):
                            nc.tensor.matmul(
                                psad[:, j, :],
                                lhsT=xo[:, k, j * 128:(j + 1) * 128],
                                rhs=w1_sb[:, k, 72:80],
                                start=(k == 0), stop=(k == 1))
                    nc.scalar.copy(out=ad1own[:, 1 + j0:1 + j0 + JB, :],
                                   in_=psad[:])
                dall1 = pat1.tile([128, 2, NW, 8], f16, tag="dall")
                build_dall(ad1own, 8, dall1, ppd1)
                at_fill(dall1, at1_all, 8, pat1, ppat1)

                # replicated table transform (emitted after at1 so the DVE
                # queue holds only at1 work; evictions go to Act only)
                AB = 10                      # chunks per DMA batch
                for cb in range(8):
                    base = cb * NLOC
                    for jj in range(0, 100, AB):
                        nb = min(AB, 100 - jj)
                        xt = pa.tile([128, 2, AB * 128], f16, tag="xt")
                        for k in range(2):
                            nc.sync.dma_start(
                                xt[:, k, 0:nb * 128],
                                xTfull[k, :, base + jj * 128:
                                       base + (jj + nb) * 128])
                        row = pa.tile([128, AB, 128], f16, tag="row")
                        for u in range(0, nb, 4):
                            ub = min(4, nb - u)
                            ps = ppa.tile([128, 4, 80], f32, tag="np1")
                            for j in range(ub):
                                for k in range(2):
                                    nc.tensor.matmul(
                                        ps[:, j, :],
                                        lhsT=xt[:, k, (u + j) * 128:
                                                (u + j + 1) * 128],
                                        rhs=w1_sb[:, k, :], start=(k == 0),
                                        stop=(k == 1))
                            nc.scalar.copy(out=row[:, u:u + ub, 0:80],
                                           in_=ps[:, 0:ub, :])
                        nc.sync.dma_start(
                            bass.AP(tab1q[cb // 2],
                                    ((cb % 2) * NLOC + jj) * 128,
                                    [[NW * 128, 128], [256, nb // 2],
                                     [1, 256]]),
                            row[:, 0:nb, :])



            # ---------- phase B: layer-1 edge pass ----------
            def edge_pass(layer, pb_ext=None, ph_ext=None):
                if layer == 1:
                    ncols_h, as_col = 72, 64
                    nheads, msgw = 8, 72
                else:
                    ncols_h, as_col = 41, 40
                    nheads, msgw = 1, 41
                pool_name = f"pe{layer}"
                import contextlib
                pbc = (contextlib.nullcontext(pb_ext) if pb_ext is not None
                       else tc.tile_pool(name=pool_name, bufs=2))
                phc = (contextlib.nullcontext(ph_ext) if ph_ext is not None
                       else tc.tile_pool(name=pool_name + "h", bufs=2))
                with (pbc as pb,
                      phc as ph,
                      tc.tile_pool(name=pool_name + "m",
                                   bufs=(1 if layer == 1 else 2)) as pm,
                      tc.tile_pool(name=pool_name + "w",
                                   bufs=(2 if layer == 1 else 3)) as pw,
                      tc.tile_pool(name=pool_name + "p", bufs=2,
                                   space="PSUM") as ppb):
                    qn = 0

                    def gather_sq(s, q, hix, ht):
                        nonlocal qn
                        k0, _ = sb_chunks[s]
                        for (ss, qq, slot0, nids) in plan["hcalls"]:
                            if ss != s or qq != q:
                                continue
                            c0 = slot0 // 128 - k0
                            if layer == 1:
                                tsl = tab1q[q][:, 0:ncols_h]
                            else:
                                tsl = ago[q * QS:(q + 1) * QS, 0:ncols_h]
                            _gather_small(
                                nc.gpsimd,
                                ht[:, c0:c0 + nids // 128, :],
                                tsl,
                                hix[:, (slot0 - k0 * 128) // 16:
                                    (slot0 - k0 * 128 + nids) // 16],
                                nids, ncols_h, 128, queue_num=qn % 4)
                            qn += 1

                    pre = l1pre if layer == 1 else {}
                    for s in range(NSB):
                        k0, k1 = sb_chunks[s]
                        cps = k1 - k0
                        if s in pre:
                            hix, ht = pre[s]
                        else:
                            hix = pb.tile([128, CPSB_MAX * 8], i16,
                                          tag="hix")
                            nc.sync.dma_start(hix[:, 0:cps * 8],
                                              hidx_d[:, k0 * 8:k1 * 8])
                            ht = ph.tile([128, CPSB_MAX, ncols_h], f16,
                                         tag="ht")
                            for q in range(NQ):
                                gather_sq(s, q, hix, ht)
                        if layer == 1:
                            at_s = at1_all[:, k0:k1, :]
                        else:
                            at_s = at2_all[:, k0:k1, 0:1]

                        e = pm.tile([128, CPSB_MAX, nheads], f32, tag="e")
                        lr = e
                        nc.vector.tensor_tensor(
                            out=e[:, 0:cps, :],
                            in0=ht[:, 0:cps, as_col:as_col + nheads],
                            in1=at_s, op=OP.add)
                        nc.vector.scalar_tensor_tensor(
                            out=lr[:, 0:cps, :], in0=e[:, 0:cps, :],
                            scalar=0.2, in1=e[:, 0:cps, :],
                            op0=OP.mult, op1=OP.max)

                        msg = pm.tile([128, CPSB_MAX, msgw], bf16, tag="msg")
                        # w into msg's trailing cols (compact exp)
                        nc.scalar.activation(
                            out=msg[:, 0:cps, as_col:as_col + nheads],
                            in_=lr[:, 0:cps, :], func=AF.Exp)
                        if layer == 1:
                            # expanded weights for a clean 2x-mode mult
                            half = (CPSB_MAX + 1) // 2
                            wgx = pm.tile([128, half, 8, 8], bf16, tag="wgx")
                            for h0 in (0, half):
                                hn = min(half, cps - h0)
                                if hn <= 0:
                                    continue
                                lrs = lr[:, h0:h0 + hn, :]
                                nc.scalar.activation(
                                    out=wgx[:, 0:hn, :, :],
                                    in_=BC(lrs, [lrs.ap[0], lrs.ap[1],
                                                 lrs.ap[2], [0, 8]]),
                                    func=AF.Exp)
                                m_ = msg[:, h0:h0 + hn, 0:64]
                                h_ = ht[:, h0:h0 + hn, 0:64]
                                nc.vector.tensor_tensor(
                                    out=BC(m_, [m_.ap[0], m_.ap[1],
                                                [8, 8], [1, 8]]),
                                    in0=BC(h_, [h_.ap[0], h_.ap[1],
                                                [8, 8], [1, 8]]),
                                    in1=wgx[:, 0:hn, :, :], op=OP.mult)
                        else:
                            wgx2 = pw.tile([128, CPSB_MAX, 40], bf16,
                                           tag="wgx2")
                            lrs = lr[:, 0:cps, :]
                            nc.scalar.activation(
                                out=wgx2[:, 0:cps, :],
                                in_=BC(lrs, [lrs.ap[0], lrs.ap[1], [0, 40]]),
                                func=AF.Exp)
                            nc.vector.tensor_tensor(
                                out=msg[:, 0:cps, 0:40],
                                in0=ht[:, 0:cps, 0:40],
                                in1=wgx2[:, 0:cps, :], op=OP.mult)

                        # windows: one-hot + aggregation matmuls, PSUM
                        # evicted into a per-sb batch tile
                        hsb = pm.tile([128, NWSB, msgw], f32, tag="hsb")
                        for wi in range(NWSB):
                            w = s * NWSB + wi
                            cols = wcols[w]
                            cpw = len(cols)
                            c0 = colbase[w]
                            ohT = pw.tile([128, 128, MAXCPW], bf16, tag="ohT")
                            dr = drel_sb[:, c0:c0 + cpw]
                            nc.vector.tensor_tensor(
                                out=ohT[:, :, 0:cpw],
                                in0=BC(dr, [dr.ap[0], [0, 128], dr.ap[1]]),
                                in1=iota_rep[:, :, 0:cpw], op=OP.is_equal)
                            ps = ppb.tile([128, msgw], f32, tag="agg")
                            for i, k in enumerate(cols):
                                nc.tensor.matmul(
                                    ps[:], lhsT=ohT[:, :, i],
                                    rhs=msg[:, k - k0, :],
                                    start=(i == 0), stop=(i == cpw - 1))
                            nc.scalar.copy(out=hsb[:, wi, :], in_=ps[:])

                        # per-sb batched softmax-normalize (+ elu/r2 for L1)
                        if layer == 1:
                            den = pw.tile([128, NWSB, 8], f32, tag="den")
                            nc.scalar.activation(out=den[:],
                                                 in_=hsb[:, :, 64:72],
                                                 func=AF.Copy, bias=ACC_EPS)
                            rec = pw.tile([128, NWSB, 8], f32, tag="rec")
                            nc.vector.reciprocal(
                                rec[:].rearrange("p a b -> p (a b)"),
                                den[:].rearrange("p a b -> p (a b)"))
                            o1 = pw.tile([128, NWSB, 64], f32, tag="o1")
                            nu = hsb[:, :, 0:64]
                            r_ = rec[:]
                            nc.vector.tensor_tensor(
                                out=BC(o1[:], [o1[:].ap[0], [64, NWSB],
                                               [8, 8], [1, 8]]),
                                in0=BC(nu, [nu.ap[0], [72, NWSB],
                                            [8, 8], [1, 8]]),
                                in1=BC(r_, [r_.ap[0], [8, NWSB],
                                            [1, 8], [0, 8]]),
                                op=OP.mult)
                            o1v = o1[:].rearrange("p a b -> p (a b)")
                            if not plan["skip_b1"]:
                                b1w = b1r[:]
                                nc.vector.tensor_tensor(
                                    out=o1v,
                                    in0=o1v,
                                    in1=BC(b1w, [b1w.ap[0], [0, NWSB],
                                                 [1, 64]]),
                                    op=OP.add)
                            # elu = relu(x) + exp(-relu(-x)) - 1
                            rneg = pw.tile([128, NWSB, 64], f32, tag="rneg")
                            nc.scalar.activation(
                                out=rneg[:].rearrange("p a b -> p (a b)"),
                                in_=o1v, func=AF.Relu, scale=-1.0)
                            expn = rneg
                            nc.scalar.activation(
                                out=expn[:].rearrange("p a b -> p (a b)"),
                                in_=rneg[:].rearrange("p a b -> p (a b)"),
                                func=AF.Exp, scale=-1.0)
                            pos = pw.tile([128, NWSB, 64], f32, tag="pos")
                            nc.scalar.activation(
                                out=pos[:].rearrange("p a b -> p (a b)"),
                                in_=o1v, func=AF.Relu)
                            hl16 = pw.tile([128, NWSB, 64], f16, tag="hl16")
                            nc.vector.scalar_tensor_tensor(
                                out=hl16[:].rearrange("p a b -> p (a b)"),
                                in0=expn[:].rearrange("p a b -> p (a b)"),
                                scalar=-1.0,
                                in1=pos[:].rearrange("p a b -> p (a b)"),
                                op0=OP.add, op1=OP.add)
                            r2s = pw.tile([128, NWSB, 42], f16, tag="r2s")
                            for wi in range(NWSB):
                                pst = ppb.tile([64, 128], f16, tag="tr")
                                nc.tensor.transpose(out=pst[:],
                                                    in_=hl16[:, wi, :],
                                                    identity=ident[:])
                                hlT = pw.tile([64, 128], f16, tag="hlT")
                                nc.scalar.copy(out=hlT[:], in_=pst[:])
                                r2p = ppb.tile([128, 42], f32, tag="r2p")
                                nc.tensor.matmul(r2p[:], lhsT=hlT[:],
                                                 rhs=w2_sb[:], start=True,
                                                 stop=True)
                                nc.scalar.copy(out=r2s[:, wi, :], in_=r2p[:])
                            nc.sync.dma_start(
                                bass.AP(agi2, (s * NWSB) * 128,
                                        [[NW * 128, 128], [128, NWSB],
                                         [1, 42]]),
                                r2s[:])
                            # stash a_dst2 (col 41) for the L2 telescoping
                            nc.scalar.copy(
                                out=ad2own[:, 1 + s * NWSB:
                                           1 + (s + 1) * NWSB, :],
                                in_=r2s[:, :, 41:42])
                        else:
                            den = pw.tile([128, NWSB, 1], f32, tag="den2")
                            nc.scalar.activation(out=den[:],
                                                 in_=hsb[:, :, 40:41],
                                                 func=AF.Copy, bias=ACC_EPS)
                            rec = pw.tile([128, NWSB, 1], f32, tag="rec2")
                            nc.vector.reciprocal(
                                rec[:].rearrange("p a b -> p (a b)"),
                                den[:].rearrange("p a b -> p (a b)"))
                            o2 = pw.tile([128, NWSB, 40], f32, tag="o2")
                            nu = hsb[:, :, 0:40]
                            r_ = rec[:]
                            nc.vector.tensor_tensor(
                                out=o2[:],
                                in0=BC(nu, [nu.ap[0], [41, NWSB], [1, 40]]),
                                in1=BC(r_, [r_.ap[0], [1, NWSB], [0, 40]]),
                                op=OP.mult)
                            o2v = o2[:].rearrange("p a b -> p (a b)")
                            if not plan["skip_b2"]:
                                b2w = b2r[:]
                                nc.vector.tensor_tensor(
                                    out=o2v, in0=o2v,
                                    in1=BC(b2w, [b2w.ap[0], [0, NWSB],
                                                 [1, 40]]),
                                    op=OP.add)
                            nc.sync.dma_start(
                                bass.AP(out, (s * NWSB) * 128 * 40,
                                        [[40, 128], [128 * 40, NWSB],
                                         [1, 40]]),
                                o2[:])

            edge_pass(1, pb_ext=pb1, ph_ext=ph1)
            estack.close()

            # ---------- AllGather first; the at2 telescoping (local data
            # only) runs during the collective ----------
            nc.gpsimd.collective_compute(
                "AllGather", OP.bypass, ins=[agi2[:]], outs=[ago[:]],
                replica_groups=[list(range(8))])

            with (tc.tile_pool(name="pat2", bufs=2) as pat2,
                  tc.tile_pool(name="ppat2", bufs=2, space="PSUM") as ppat2,
                  tc.tile_pool(name="ppd2", bufs=1, space="PSUM") as ppd2):
                dall2 = pat2.tile([128, 2, NW, 1], f16, tag="dall")
                build_dall(ad2own, 1, dall2, ppd2)
                at_fill(dall2, at2_all, 1, pat2, ppat2)

            edge_pass(2)

    nc.finalize()
    return nc


def kernel(**inputs):
    per_core, plan = _host_prep(**inputs)
    if "nc" not in _CACHE:
        _CACHE["nc"] = _build_nc(plan)
    nc = _CACHE["nc"]
    from concourse.bass_utils import run_bass_kernel_spmd
    res = run_bass_kernel_spmd(nc, per_core, list(range(8)))
    full = np.concatenate([res.results[c]["out"] for c in range(8)], axis=0)
    return np.ascontiguousarray(full[:N]).astype(np.float32)



# revision 86
# speedup vs baseline: 1.0635x; 1.0027x over previous
"""GAT (2-layer PyG GATConv, eval) on 8 Trainium2 NeuronCores.

Sharding: nodes range-partitioned (NLOC=12800/core); core c owns edges whose
dst is in its range. The layer-1 node table is REPLICATED: every core
transforms all 102400 rows from the full x (the x stream + table write cost
less on the DMA timeline than the AllGather they replace, and the a_dst
telescoping runs concurrently on DVE from the core's own x shard). Layer 2's
table still needs one AllGather (agi2 -> ago), overlapped by the at2
telescoping machinery.

Slot layout per core: superblock (10 windows) -> quadrant -> window, with
per-(window,quadrant) STATIC capacities = max edge count over the 8 cores
(SPMD: one module runs on all cores; only tensor contents differ) — ~10%
slot padding vs 28% for fixed-size groups. Gather calls use
single_packet=False, which lets the SWDGE ucode stream descriptors through
the ring: up to 4x the ring (dynamic_dma_scratch_size/16) indices per call
(probed on HW: 8192 idx streams through a 2048-desc ring; 8x crashes;
single-packet calls hard-crash above 1024 idx). One ~5.5k-idx call per
(sb,quadrant) segment cuts the 994ns-per-call Pool overhead ~5x vs the
1024-idx baseline.

Per layer, per edge slot: a 144B/82B payload gather pulls [h|a_src] rows
(256B-stride tables, int16 idx into 25600-row quadrants). a_dst is NOT
gathered: slots are drel-sorted inside each (window,quadrant) group, so
per-slot a_dst[drel] telescopes — P[d,s] = (s >= start[d]) is a DVE is_ge
step matrix (host-static start tables) and PE computes P^T @ diff(a_dst)
per chunk column; window starts mid-chunk use a compensated diff table
(row0 = ad[0,w]-ad[127,w-1]) accumulated in the same PSUM. Both at1 and at2
expansions run during their layer's AllGather (they read only local data).
Softmax is the shift-invariant no-max form (w = exp(leakyrelu(as+ad)),
|e| < ~25 so fp32 exp is safe). Segment reduction is a PE matmul whose
stationary matrix is a transposed one-hot built by DVE is_equal in the
2x-mode layout; boundary chunks carry a masked drel column per touching
window. msg = h*w uses an Act-expanded weight tile so the DVE mult runs in
2x mode. Softmax normalize + ELU + the r2 = hlT @ [W2|a2_src|a2_dst]
projection are batched per superblock; tables are written node-permuted
(row = p*100 + j within each core block) so writes coalesce per partition.
"""
import numpy as np
import ml_dtypes

N = 100000
E = 1600000
NF = 256
HEADS, NHID = 8, 8
NH = HEADS * NHID          # 64
NCLASS = 40
NLOC = 12800               # nodes per core
NW = 100                   # 128-dst windows per core
NQ = 4                     # src table quadrants
QS = 25600                 # rows per quadrant
NWSB = 10                  # windows per superblock
NSB = NW // NWSB           # 10 superblocks
NTOT = 102400
ACC_EPS = 1e-16

_CACHE = {}


def _ceil128(x):
    return (x + 127) & ~127


def _host_prep(x, edge_index, W1, a1_src, a1_dst, b1, W2, a2_src, a2_dst, b2):
    src = np.asarray(edge_index[0], dtype=np.int64)
    dst = np.asarray(edge_index[1], dtype=np.int64)

    # table-row permutation: node n -> row  c*NLOC + (l%128)*NW + l//128
    def rowperm(n):
        c = n // NLOC
        l = n - c * NLOC
        return c * NLOC + (l % 128) * NW + l // 128

    srow = rowperm(src)
    sq = srow // QS
    sidx = (srow - sq * QS).astype(np.int16)

    core = dst // NLOC
    dloc = dst - core * NLOC
    w_e = (dloc >> 7).astype(np.int64)
    dr_e = (dloc & 127).astype(np.int64)
    adidx = (dr_e * NW + w_e).astype(np.int16)
    sb_e = w_e // NWSB

    # static capacities: max over cores per (window, quadrant); >=1 so every
    # group is present in the slot stream (the telescoped a_dst expansion
    # needs window w-1 to precede window w inside each (sb,q) segment)
    gkey = (core * NW + w_e) * NQ + sq          # [E]
    cnt = np.bincount(gkey, minlength=8 * NW * NQ).reshape(8, NW, NQ)
    cap = np.maximum(cnt.max(axis=0), 1)         # [NW, NQ]

    # slot layout: sb -> quadrant -> window.  With single_packet=False the
    # SWDGE gather ucode streams descriptors through the ring, so calls up
    # to 4x the ring size (dynamic_dma_scratch_size/16) are fine (probed on
    # HW: 8192 idx streams through a 2048-desc ring; 8x crashes).
    MAXIDX = 8192
    wq_start = np.zeros((NW, NQ), np.int64)
    hcalls = []                                  # (sb, q, slot0, n_idx)
    sb_chunks = []                               # (k0, k1) per sb
    nslot = 0
    for s in range(NSB):
        k0 = nslot // 128
        for q in range(NQ):
            seg0 = nslot
            for w in range(s * NWSB, (s + 1) * NWSB):
                wq_start[w, q] = nslot
                nslot += int(cap[w, q])
            nslot = _ceil128(nslot)
            for off in range(seg0, nslot, MAXIDX):
                hcalls.append((s, q, off, min(MAXIDX, nslot - off)))
        sb_chunks.append((k0, nslot // 128))
    NSLOT = nslot
    NCHUNK = NSLOT // 128

    # window label per slot (shared across cores: layout is static).  Group
    # pads inherit their group's window; (sb,q)-tail ceil128 pads inherit the
    # segment's last window.
    wfull = np.zeros(NSLOT, np.int64)
    for s in range(NSB):
        for q in range(NQ):
            for w in range(s * NWSB, (s + 1) * NWSB):
                a = int(wq_start[w, q])
                e_ = a + int(cap[w, q])
                wfull[a:e_] = w
            wfull[e_:_ceil128(e_)] = (s + 1) * NWSB - 1   # segment tail pads

    # at-plan: per chunk, one column-copy per touching window; copies of one
    # chunk accumulate in PSUM.  v=1 (compensated diff table) iff the window
    # starts mid-chunk.  Packed into segments of <= ATSEGC columns.
    ATSEGC = 24
    atsegs = []            # (kg0, nk, c0, cols=[(i, klocal, v, w, first, last)])
    copies_per_chunk = []  # [(w, a, b)]
    for k in range(NCHUNK):
        wk = wfull[k * 128:(k + 1) * 128]
        runs = []
        a = 0
        for p in range(1, 128):
            if wk[p] != wk[p - 1]:
                runs.append((int(wk[a]), a, p))
                a = p
        runs.append((int(wk[a]), a, 128))
        copies_per_chunk.append(runs)
    ncolat = 0
    cur = None
    colat_of = {}
    for k in range(NCHUNK):
        runs = copies_per_chunk[k]
        if cur is None or cur[3] + len(runs) > ATSEGC:
            if cur is not None:
                atsegs.append(cur)
            cur = [k, 0, ncolat, 0, []]
        kloc = cur[1]
        for i, (w, a, b) in enumerate(runs):
            colat_of[(k, a)] = ncolat
            cur[4].append((cur[3], kloc, 1 if a > 0 else 0, w,
                           i == 0, i == len(runs) - 1))
            cur[3] += 1
            ncolat += 1
        cur[1] += 1
    if cur is not None:
        atsegs.append(cur)
    NCOLAT = ncolat

    # per-window chunk columns
    colmap = np.full((NW, NCHUNK), -1, np.int64)
    wcols = []                                   # per w: (colbase, [chunks])
    ncol = 0
    for w in range(NW):
        cols = []
        for q in range(NQ):
            a = int(wq_start[w, q])
            b = a + int(cap[w, q])
            for k in range(a // 128, (b + 127) // 128):
                cols.append(k)
                colmap[w, k] = ncol
                ncol += 1
        wcols.append(cols)
    NCOL = ncol
    MAXCPW = max(len(c) for c in wcols)

    plan = {
        "NSLOT": NSLOT, "NCHUNK": NCHUNK, "NCOL": NCOL, "MAXCPW": MAXCPW,
        "NCOLAT": NCOLAT, "ATSEGC": ATSEGC,
        "hcalls": hcalls, "atsegs": atsegs, "sb_chunks": sb_chunks,
        "wcols": wcols,
        "skip_b1": bool(np.all(np.asarray(b1) == 0)),
        "skip_b2": bool(np.all(np.asarray(b2) == 0)),
    }

    # group-id in slot order: (sb, q, w_in_sb)
    flatg = (sb_e * NQ + sq) * NWSB + (w_e - sb_e * NWSB)
    gstart_flat = np.zeros(NSB * NQ * NWSB, np.int64)
    for s in range(NSB):
        for q in range(NQ):
            for wi in range(NWSB):
                gstart_flat[(s * NQ + q) * NWSB + wi] = wq_start[s * NWSB + wi, q]

    per_core = []
    hidx_all, startd_all, drel_all = [], [], []
    for c in range(8):
        m = core == c
        fg = flatg[m]
        drc = dr_e[m]
        order = np.lexsort((drc, fg))
        fgs = fg[order]
        cntc = np.bincount(fgs, minlength=NSB * NQ * NWSB)
        starts = np.zeros_like(cntc)
        starts[1:] = np.cumsum(cntc)[:-1]
        rank = np.arange(len(fgs)) - starts[fgs]
        slot = gstart_flat[fgs] + rank

        hvec = np.zeros(NSLOT, np.int16)
        hvec[slot] = sidx[m][order]

        drel = np.full((128, NCOL), 128.0, np.float32)
        k_s = slot >> 7
        p_s = slot & 127
        we_s = w_e[m][order]
        col_s = colmap[we_s, k_s]
        assert (col_s >= 0).all()
        drel[p_s, col_s] = drc[order].astype(np.float32)

        # per-slot drel stream (pads = 128) for the telescoped start tables
        drfull = np.full(NSLOT, 128, np.int64)
        drfull[slot] = drc[order]
        startd = np.zeros((128, NCOLAT), np.float32)
        dgrid = np.arange(128)
        for k in range(NCHUNK):
            for (w, a, b) in copies_per_chunk[k]:
                col = colat_of[(k, a)]
                drs = drfull[k * 128 + a:k * 128 + b]
                startd[:, col] = a + np.searchsorted(drs, dgrid)

        def wrap16(v):
            o = np.zeros((128, NSLOT // 16), np.int16)
            sl = np.arange(NSLOT)
            o[sl % 16, sl // 16] = v
            for r in range(1, 8):
                o[16 * r:16 * (r + 1)] = o[:16]
            return o

        hidx_all.append(wrap16(hvec))
        startd_all.append(startd.astype(np.float16))
        drel_all.append(drel.astype(ml_dtypes.bfloat16))

    # weights
    W1 = np.asarray(W1, np.float32)
    v_s1 = np.einsum("chk,hk->ch", W1.reshape(NF, HEADS, NHID),
                     np.asarray(a1_src, np.float32))
    v_d1 = np.einsum("chk,hk->ch", W1.reshape(NF, HEADS, NHID),
                     np.asarray(a1_dst, np.float32))
    W1e = np.concatenate([W1, v_s1, v_d1], axis=1).reshape(2, 128, 80)
    W1e = W1e.astype(np.float16)

    W2 = np.asarray(W2, np.float32)
    v_s2 = W2 @ np.asarray(a2_src, np.float32)[0]
    v_d2 = W2 @ np.asarray(a2_dst, np.float32)[0]
    W2e = np.concatenate([W2, v_s2[:, None], v_d2[:, None]],
                         axis=1).astype(np.float16)   # [64, 42]

    xp = np.zeros((NTOT, NF), np.float32)
    xp[:N] = np.asarray(x, np.float32)

    # lhsT matrices for building the diff tables on PE:
    # mshift[:,0,:] = Mplain^T (fwd diff), mshift[:,1,:] = -sel(127)->row0
    mshiftT = np.zeros((128, 2, 128), np.float16)
    mshiftT[:, 0, :] = (np.eye(128) - np.eye(128, k=1)).astype(np.float16)
    mshiftT[127, 1, 0] = -1.0

    # full transposed x, shared by all cores: the layer-1 table transform is
    # replicated (each core computes all 102400 rows locally; the x stream +
    # table write fit under what the AllGather used to cost, and the a_dst
    # telescoping runs concurrently from the core's own x shard)
    xT = np.ascontiguousarray(xp.T).astype(np.float16).reshape(2, 128, NTOT)

    for c in range(8):
        xloc = np.ascontiguousarray(xp[c * NLOC:(c + 1) * NLOC].T)
        per_core.append({
            "xTloc": xloc.astype(np.float16).reshape(2, 128, NLOC),
            "xTfull": xT,
            "W1e": W1e,
            "W2e": W2e,
            "b1": np.asarray(b1, np.float32)[None, :],
            "b2": np.asarray(b2, np.float32)[None, :],
            "hidx": hidx_all[c],
            "startd": startd_all[c],
            "drel": drel_all[c],
            "mshift": mshiftT,
        })
    return per_core, plan


def _gather_small(g, out_ap, in_ap, idxs_ap, num_idxs, elem_size, elem_step,
                  queue_num=0, single_packet=False):
    """dma_gather with payload < 256B; only the 256B row-stride rule is real
    for the non-transpose path."""
    import concourse.mybir as mybir
    stride_bytes = elem_step * mybir.dt.size(in_ap.dtype)
    assert stride_bytes % 256 == 0
    _in_ap = g.lower_ap_dma(in_ap, for_custom_bir_dma=True)
    _idxs_ap = g.lower_ap(idxs_ap)
    _out_ap = g.lower_ap(out_ap)
    return g.add_instruction(mybir.InstDMAGatherAnt(
        name=g.bass.get_next_instruction_name(),
        ins=[*_in_ap, _idxs_ap, g.lower_val_access(g.to_reg(num_idxs))],
        outs=[_out_ap],
        transpose=False,
        num_idxs=num_idxs,
        elem_size=elem_size,
        stride_bytes_256=stride_bytes // 256,
        gen_mode=0,
        single_packet=single_packet,
        queue_num=queue_num,
        sbuf_tokens_per_rank=0,
        sbuf_free_dim_per_rank=0,
        sbuf_free_dim_pad_per_rank=0,
        sbuf_byte_offset=0,
    ))


def _build_nc(plan):
    import concourse.bass as bass
    import concourse.bacc as bacc
    import concourse.mybir as mybir
    import concourse.tile as tile
    from concourse.library_config import mlp
    from concourse.masks import make_identity

    f32, f16, bf16, i16 = (mybir.dt.float32, mybir.dt.float16,
                           mybir.dt.bfloat16, mybir.dt.int16)
    AF = mybir.ActivationFunctionType
    OP = mybir.AluOpType

    NSLOT = plan["NSLOT"]
    NCOL = plan["NCOL"]
    MAXCPW = plan["MAXCPW"]
    sb_chunks = plan["sb_chunks"]
    wcols = plan["wcols"]
    CPSB_MAX = max(k1 - k0 for k0, k1 in sb_chunks)
    colbase = [0] * NW
    for w in range(1, NW):
        colbase[w] = colbase[w - 1] + len(wcols[w - 1])

    NCOLAT = plan["NCOLAT"]
    ATSEGC = plan["ATSEGC"]
    atsegs = plan["atsegs"]

    nc = bacc.Bacc("TRN2", target_bir_lowering=False, debug=False,
                   num_devices=8, num_swdge_queues=4,
                   dynamic_dma_scratch_size=32768)

    xTloc = nc.dram_tensor("xTloc", [2, 128, NLOC], f16, kind="ExternalInput")
    xTfull = nc.dram_tensor("xTfull", [2, 128, NTOT], f16,
                            kind="ExternalInput")
    W1e = nc.dram_tensor("W1e", [2, 128, 80], f16, kind="ExternalInput")
    W2e = nc.dram_tensor("W2e", [64, 42], f16, kind="ExternalInput")
    b1 = nc.dram_tensor("b1", [1, 64], f32, kind="ExternalInput")
    b2 = nc.dram_tensor("b2", [1, 40], f32, kind="ExternalInput")
    hidx_d = nc.dram_tensor("hidx", [128, NSLOT // 16], i16,
                            kind="ExternalInput")
    startd_d = nc.dram_tensor("startd", [128, NCOLAT], f16,
                              kind="ExternalInput")
    mshift_d = nc.dram_tensor("mshift", [128, 2, 128], f16,
                              kind="ExternalInput")
    drel_d = nc.dram_tensor("drel", [128, NCOL], bf16, kind="ExternalInput")
    out = nc.dram_tensor("out", [NLOC, 40], f32, kind="ExternalOutput")

    # per-QUADRANT table tensors: (sb,q) gathers depend only on quadrant q's
    # transform writes, so the gather stream overlaps the transform tail
    tab1q = [nc.dram_tensor(f"tab1q{q}", [QS, 128], f16) for q in range(4)]
    agi2 = nc.dram_tensor("agi2", [NLOC, 128], f16)    # [h2|as2|ad2|pad]
    ago = nc.dram_tensor("ago", [NTOT, 128], f16, addr_space="Shared")

    def BC(ap, dims):
        return bass.AP(ap.tensor, ap.offset, dims)

    def dram_rows(t, offset_rows, dims):
        """AP into DRAM tensor t (row-major, 128 f16 cols) at row offset."""
        return bass.AP(t, offset_rows * 128, dims)

    with tile.TileContext(nc) as tc:
        with tc.tile_pool(name="const", bufs=1) as pc:
            nc.gpsimd.load_library(mlp)

            drel_sb = pc.tile([128, NCOL], bf16)
            nc.sync.dma_start(drel_sb[:], drel_d[:])
            w1_sb = pc.tile([128, 2, 80], f16)
            nc.sync.dma_start(w1_sb[:], W1e[:].rearrange("k p n -> p k n"))
            w2_sb = pc.tile([64, 42], f16)
            nc.sync.dma_start(w2_sb[:], W2e[:])

            NIOTA = max(MAXCPW, ATSEGC)
            iota_rep = pc.tile([128, 128, MAXCPW], bf16)
            iota_at = pc.tile([128, 128, ATSEGC], f16)
            with tc.tile_pool(name="pii", bufs=1) as pii:
                ii = pii.tile([128, 128, NIOTA], i16)
                nc.gpsimd.iota(ii[:], pattern=[[1, 128], [0, NIOTA]],
                               base=0, channel_multiplier=0)
                nc.vector.tensor_copy(out=iota_rep[:],
                                      in_=ii[:, :, 0:MAXCPW])
                nc.vector.tensor_copy(out=iota_at[:],
                                      in_=ii[:, :, 0:ATSEGC])
            mshift_sb = pc.tile([128, 2, 128], f16)
            nc.sync.dma_start(mshift_sb[:], mshift_d[:])

            ident = pc.tile([128, 128], f16)
            make_identity(nc, ident[:])

            ones32 = pc.tile([1, 128], f32)
            nc.vector.memset(ones32[:], 1.0)

            b1r = pc.tile([128, 64], f32)
            b2r = pc.tile([128, 40], f32)
            with tc.tile_pool(name="pini", bufs=2, space="PSUM") as ppi:
                for row_d, width, dest in ((b1, 64, b1r), (b2, 40, b2r)):
                    t = pc.tile([1, width], f32, tag=f"rrow{width}")
                    nc.sync.dma_start(t[:], row_d[:])
                    ps = ppi.tile([128, width], f32, tag="rep")
                    nc.tensor.matmul(ps[:], lhsT=ones32[:], rhs=t[:],
                                     start=True, stop=True)
                    nc.vector.tensor_copy(out=dest[:], in_=ps[:])

            # a_dst per-slot tiles, telescoped from the own-block a_dst
            # values via PE (P[d,s] = (s >= start[d]) is a step matrix;
            # P @ diff(ad) = ad[drel[s]] since slots are drel-sorted)
            at1_all = pc.tile([128, plan["NCHUNK"], 8], f16)
            at2_all = pc.tile([128, plan["NCHUNK"], 1], f16)
            ad2own = pc.tile([128, NW + 1, 1], f16)
            nc.vector.memset(ad2own[:, 0:1, :].rearrange("p a b -> p (a b)"),
                             0.0)

            def at_fill(dall, at_tile, H, pat, ppat):
                # per-scope start-table load (keeps it out of the const pool,
                # freeing SBUF for the edge-pass msg double-buffer)
                startd_sb = pat.tile([128, NCOLAT], f16, tag="startd")
                nc.sync.dma_start(startd_sb[:], startd_d[:])
                for (kg0, nk, c0, ncols, cols) in atsegs:
                    P = pat.tile([128, 128, ATSEGC], f16, tag="P")
                    st_ = startd_sb[:, c0:c0 + ncols]
                    nc.vector.tensor_tensor(
                        out=P[:, :, 0:ncols],
                        in0=iota_at[:, :, 0:ncols],
                        in1=BC(st_, [st_.ap[0], [0, 128], st_.ap[1]]),
                        op=OP.is_ge)
                    ps = ppat.tile([128, ATSEGC, 8], f32, tag="atps")
                    for (i, klocal, v, w, first, last) in cols:
                        nc.tensor.matmul(ps[:, klocal, 0:H],
                                         lhsT=P[:, :, i],
                                         rhs=dall[:, v, w, 0:H],
                                         start=first, stop=last)
                    nc.scalar.activation(out=at_tile[:, kg0:kg0 + nk, :],
                                         in_=ps[:, 0:nk, 0:H], func=AF.Copy)

            def build_dall(adown, H, dall, ppd):
                # adown: [128, NW+1, H] f16, col 0 zeroed.
                # dall[:,0,w,:] = fwd-diff (row d: ad[d]-ad[d-1], row0 ad[0]),
                # dall[:,1,w,:] = same but row0 = ad[0,w]-ad[127,w-1].
                # PSUM rows padded to whole banks (512 f32); each matmul's
                # output chunk must stay inside one bank.  The two diff
                # variants run sequentially through one 2-bank tile.
                psrow = ((NW * H + 511) // 512) * 512
                a_ = adown[:]
                for v in range(2):
                    ps = ppd.tile([128, psrow], f32, tag=f"dps{H}")
                    for off in range(0, NW * H, 512):
                        nn = min(512, NW * H - off)
                        rhs_cur = bass.AP(a_.tensor, a_.offset + H + off,
                                          [a_.ap[0], [1, nn]])
                        rhs_prev = bass.AP(a_.tensor, a_.offset + off,
                                           [a_.ap[0], [1, nn]])
                        nc.tensor.matmul(ps[:, off:off + nn],
                                         lhsT=mshift_sb[:, 0, :],
                                         rhs=rhs_cur, start=True,
                                         stop=(v == 0))
                        if v == 1:
                            nc.tensor.matmul(ps[:, off:off + nn],
                                             lhsT=mshift_sb[:, 1, :],
                                             rhs=rhs_prev, start=False,
                                             stop=True)
                    nc.scalar.activation(
                        out=dall[:, v, :, :].rearrange("p b c -> p (b c)"),
                        in_=ps[:, 0:NW * H], func=AF.Copy)

            # Allocate the L1 gather pools BEFORE (under) the transform pools
            # in the stack allocator: address-disjoint tiles carry no WAR
            # dep on the transform scope, and with per-quadrant table
            # tensors the (sb,q) gathers start as soon as quadrant q lands.
            import contextlib
            estack = contextlib.ExitStack()
            pb1 = estack.enter_context(tc.tile_pool(name="pe1", bufs=2))
            ph1 = estack.enter_context(tc.tile_pool(name="pe1h", bufs=2))

            l1pre = {}

            # ---------- phase A (REPLICATED, no collective): the full-table
            # transform streams xTfull on DMA/PE/Act while, concurrently,
            # DVE runs the at1 telescoping.  ad1own = x_own @ v_d1 is
            # computed directly from the core's own x shard (same fused math
            # as the table's a_dst columns), so at1 never touches tab1;
            # engines are disjoint: DMA+PE+Act = transform, DVE+PE = at1.
            with (tc.tile_pool(name="pa", bufs=3) as pa,
                  tc.tile_pool(name="ppa", bufs=2, space="PSUM") as ppa,
                  tc.tile_pool(name="pat1", bufs=2) as pat1,
                  tc.tile_pool(name="ppat1", bufs=2, space="PSUM") as ppat1,
                  tc.tile_pool(name="ppd1", bufs=1, space="PSUM") as ppd1):
                # ad1own: per j-block matmul x_own[:, j] @ v_d1 (cols 72:80
                # of W1e), accumulated over the two k-halves
                ad1own = pat1.tile([128, NW + 1, 8], f16, tag="adown")
                nc.vector.memset(
                    ad1own[:, 0:1, :].rearrange("p a b -> p (a b)"), 0.0)
                JB = 10
                for j0 in range(0, NW, JB):
                    xo = pat1.tile([128, 2, JB * 128], f16, tag="xo")
                    for k in range(2):
                        nc.sync.dma_start(
                            xo[:, k, :],
                            xTloc[k, :, j0 * 128:(j0 + JB) * 128])
                    psad = ppat1.tile([128, JB, 8], f32, tag="psad")
                    for j in range(JB):
                        for k in range(2):
                            nc.tensor.matmul(
                                psad[:, j, :],
                                lhsT=xo[:, k, j * 128:(j + 1) * 128],
                                rhs=w1_sb[:, k, 72:80],
                                start=(k == 0), stop=(k == 1))
                    nc.scalar.copy(out=ad1own[:, 1 + j0:1 + j0 + JB, :],
                                   in_=psad[:])
                dall1 = pat1.tile([128, 2, NW, 8], f16, tag="dall")
                build_dall(ad1own, 8, dall1, ppd1)
                at_fill(dall1, at1_all, 8, pat1, ppat1)

                # replicated table transform (emitted after at1 so the DVE
                # queue holds only at1 work; evictions go to Act only)
                AB = 10                      # chunks per DMA batch
                for cb in range(8):
                    base = cb * NLOC
                    for jj in range(0, 100, AB):
                        nb = min(AB, 100 - jj)
                        xt = pa.tile([128, 2, AB * 128], f16, tag="xt")
                        for k in range(2):
                            nc.sync.dma_start(
                                xt[:, k, 0:nb * 128],
                                xTfull[k, :, base + jj * 128:
                                       base + (jj + nb) * 128])
                        row = pa.tile([128, AB, 128], f16, tag="row")
                        for u in range(0, nb, 4):
                            ub = min(4, nb - u)
                            ps = ppa.tile([128, 4, 80], f32, tag="np1")
                            for j in range(ub):
                                for k in range(2):
                                    nc.tensor.matmul(
                                        ps[:, j, :],
                                        lhsT=xt[:, k, (u + j) * 128:
                                                (u + j + 1) * 128],
                                        rhs=w1_sb[:, k, :], start=(k == 0),
                                        stop=(k == 1))
                            nc.scalar.copy(out=row[:, u:u + ub, 0:80],
                                           in_=ps[:, 0:ub, :])
                        nc.sync.dma_start(
                            bass.AP(tab1q[cb // 2],
                                    ((cb % 2) * NLOC + jj) * 128,
                                    [[NW * 128, 128], [256, nb // 2],
                                     [1, 256]]),
                            row[:, 0:nb, :])



            # ---------- phase B: layer-1 edge pass ----------
            def edge_pass(layer, pb_ext=None, ph_ext=None):
                if layer == 1:
                    ncols_h, as_col = 72, 64
                    nheads, msgw = 8, 72
                else:
                    ncols_h, as_col = 41, 40
                    nheads, msgw = 1, 41
                pool_name = f"pe{layer}"
                import contextlib
                pbc = (contextlib.nullcontext(pb_ext) if pb_ext is not None
                       else tc.tile_pool(name=pool_name, bufs=2))
                phc = (contextlib.nullcontext(ph_ext) if ph_ext is not None
                       else tc.tile_pool(name=pool_name + "h", bufs=3))
                with (pbc as pb,
                      phc as ph,
                      tc.tile_pool(name=pool_name + "m",
                                   bufs=(1 if layer == 1 else 2)) as pm,
                      tc.tile_pool(name=pool_name + "w",
                                   bufs=(2 if layer == 1 else 3)) as pw,
                      tc.tile_pool(name=pool_name + "p", bufs=2,
                                   space="PSUM") as ppb):
                    qn = 0

                    def gather_sq(s, q, hix, ht):
                        nonlocal qn
                        k0, _ = sb_chunks[s]
                        for (ss, qq, slot0, nids) in plan["hcalls"]:
                            if ss != s or qq != q:
                                continue
                            c0 = slot0 // 128 - k0
                            if layer == 1:
                                tsl = tab1q[q][:, 0:ncols_h]
                            else:
                                tsl = ago[q * QS:(q + 1) * QS, 0:ncols_h]
                            _gather_small(
                                nc.gpsimd,
                                ht[:, c0:c0 + nids // 128, :],
                                tsl,
                                hix[:, (slot0 - k0 * 128) // 16:
                                    (slot0 - k0 * 128 + nids) // 16],
                                nids, ncols_h, 128, queue_num=qn % 4)
                            qn += 1

                    pre = l1pre if layer == 1 else {}
                    for s in range(NSB):
                        k0, k1 = sb_chunks[s]
                        cps = k1 - k0
                        if s in pre:
                            hix, ht = pre[s]
                        else:
                            hix = pb.tile([128, CPSB_MAX * 8], i16,
                                          tag="hix")
                            nc.sync.dma_start(hix[:, 0:cps * 8],
                                              hidx_d[:, k0 * 8:k1 * 8])
                            ht = ph.tile([128, CPSB_MAX, ncols_h], f16,
                                         tag="ht")
                            for q in range(NQ):
                                gather_sq(s, q, hix, ht)
                        if layer == 1:
                            at_s = at1_all[:, k0:k1, :]
                        else:
                            at_s = at2_all[:, k0:k1, 0:1]

                        e = pm.tile([128, CPSB_MAX, nheads], f32, tag="e")
                        lr = e
                        nc.vector.tensor_tensor(
                            out=e[:, 0:cps, :],
                            in0=ht[:, 0:cps, as_col:as_col + nheads],
                            in1=at_s, op=OP.add)
                        nc.vector.scalar_tensor_tensor(
                            out=lr[:, 0:cps, :], in0=e[:, 0:cps, :],
                            scalar=0.2, in1=e[:, 0:cps, :],
                            op0=OP.mult, op1=OP.max)

                        msg = pm.tile([128, CPSB_MAX, msgw], bf16, tag="msg")
                        # w into msg's trailing cols (compact exp)
                        nc.scalar.activation(
                            out=msg[:, 0:cps, as_col:as_col + nheads],
                            in_=lr[:, 0:cps, :], func=AF.Exp)
                        if layer == 1:
                            # expanded weights for a clean 2x-mode mult
                            half = (CPSB_MAX + 1) // 2
                            wgx = pm.tile([128, half, 8, 8], bf16, tag="wgx")
                            for h0 in (0, half):
                                hn = min(half, cps - h0)
                                if hn <= 0:
                                    continue
                                lrs = lr[:, h0:h0 + hn, :]
                                nc.scalar.activation(
                                    out=wgx[:, 0:hn, :, :],
                                    in_=BC(lrs, [lrs.ap[0], lrs.ap[1],
                                                 lrs.ap[2], [0, 8]]),
                                    func=AF.Exp)
                                m_ = msg[:, h0:h0 + hn, 0:64]
                                h_ = ht[:, h0:h0 + hn, 0:64]
                                nc.vector.tensor_tensor(
                                    out=BC(m_, [m_.ap[0], m_.ap[1],
                                                [8, 8], [1, 8]]),
                                    in0=BC(h_, [h_.ap[0], h_.ap[1],
                                                [8, 8], [1, 8]]),
                                    in1=wgx[:, 0:hn, :, :], op=OP.mult)
                        else:
                            wgx2 = pw.tile([128, CPSB_MAX, 40], bf16,
                                           tag="wgx2")
                            lrs = lr[:, 0:cps, :]
                            nc.scalar.activation(
                                out=wgx2[:, 0:cps, :],
                                in_=BC(lrs, [lrs.ap[0], lrs.ap[1], [0, 40]]),
                                func=AF.Exp)
                            nc.vector.tensor_tensor(
                                out=msg[:, 0:cps, 0:40],
                                in0=ht[:, 0:cps, 0:40],
                                in1=wgx2[:, 0:cps, :], op=OP.mult)

                        # windows: one-hot + aggregation matmuls, PSUM
                        # evicted into a per-sb batch tile
                        hsb = pm.tile([128, NWSB, msgw], f32, tag="hsb")
                        for wi in range(NWSB):
                            w = s * NWSB + wi
                            cols = wcols[w]
                            cpw = len(cols)
                            c0 = colbase[w]
                            ohT = pw.tile([128, 128, MAXCPW], bf16, tag="ohT")
                            dr = drel_sb[:, c0:c0 + cpw]
                            nc.vector.tensor_tensor(
                                out=ohT[:, :, 0:cpw],
                                in0=BC(dr, [dr.ap[0], [0, 128], dr.ap[1]]),
                                in1=iota_rep[:, :, 0:cpw], op=OP.is_equal)
                            ps = ppb.tile([128, msgw], f32, tag="agg")
                            for i, k in enumerate(cols):
                                nc.tensor.matmul(
                                    ps[:], lhsT=ohT[:, :, i],
                                    rhs=msg[:, k - k0, :],
                                    start=(i == 0), stop=(i == cpw - 1))
                            nc.scalar.copy(out=hsb[:, wi, :], in_=ps[:])

                        # per-sb batched softmax-normalize (+ elu/r2 for L1)
                        if layer == 1:
                            den = pw.tile([128, NWSB, 8], f32, tag="den")
                            nc.scalar.activation(out=den[:],
                                                 in_=hsb[:, :, 64:72],
                                                 func=AF.Copy, bias=ACC_EPS)
                            rec = pw.tile([128, NWSB, 8], f32, tag="rec")
                            nc.vector.reciprocal(
                                rec[:].rearrange("p a b -> p (a b)"),
                                den[:].rearrange("p a b -> p (a b)"))
                            o1 = pw.tile([128, NWSB, 64], f32, tag="o1")
                            nu = hsb[:, :, 0:64]
                            r_ = rec[:]
                            nc.vector.tensor_tensor(
                                out=BC(o1[:], [o1[:].ap[0], [64, NWSB],
                                               [8, 8], [1, 8]]),
                                in0=BC(nu, [nu.ap[0], [72, NWSB],
                                            [8, 8], [1, 8]]),
                                in1=BC(r_, [r_.ap[0], [8, NWSB],
                                            [1, 8], [0, 8]]),
                                op=OP.mult)
                            o1v = o1[:].rearrange("p a b -> p (a b)")
                            if not plan["skip_b1"]:
                                b1w = b1r[:]
                                nc.vector.tensor_tensor(
                                    out=o1v,
                                    in0=o1v,
                                    in1=BC(b1w, [b1w.ap[0], [0, NWSB],
                                                 [1, 64]]),
                                    op=OP.add)
                            # elu = relu(x) + exp(-relu(-x)) - 1
                            rneg = pw.tile([128, NWSB, 64], f32, tag="rneg")
                            nc.scalar.activation(
                                out=rneg[:].rearrange("p a b -> p (a b)"),
                                in_=o1v, func=AF.Relu, scale=-1.0)
                            expn = rneg
                            nc.scalar.activation(
                                out=expn[:].rearrange("p a b -> p (a b)"),
                                in_=rneg[:].rearrange("p a b -> p (a b)"),
                                func=AF.Exp, scale=-1.0)
                            pos = pw.tile([128, NWSB, 64], f32, tag="pos")
                            nc.scalar.activation(
                                out=pos[:].rearrange("p a b -> p (a b)"),
                                in_=o1v, func=AF.Relu)
                            hl16 = pw.tile([128, NWSB, 64], f16, tag="hl16")
                            nc.vector.scalar_tensor_tensor(
                                out=hl16[:].rearrange("p a b -> p (a b)"),
                                in0=expn[:].rearrange("p a b -> p (a b)"),
                                scalar=-1.0,
                                in1=pos[:].rearrange("p a b -> p (a b)"),
                                op0=OP.add, op1=OP.add)
                            r2s = pw.tile([128, NWSB, 42], f16, tag="r2s")
                            for wi in range(NWSB):
                                pst = ppb.tile([64, 128], f16, tag="tr")
                                nc.tensor.transpose(out=pst[:],
                                                    in_=hl16[:, wi, :],
                                                    identity=ident[:])
                                hlT = pw.tile([64, 128], f16, tag="hlT")
                                nc.scalar.copy(out=hlT[:], in_=pst[:])
                                r2p = ppb.tile([128, 42], f32, tag="r2p")
                                nc.tensor.matmul(r2p[:], lhsT=hlT[:],
                                                 rhs=w2_sb[:], start=True,
                                                 stop=True)
                                nc.scalar.copy(out=r2s[:, wi, :], in_=r2p[:])
                            nc.sync.dma_start(
                                bass.AP(agi2, (s * NWSB) * 128,
                                        [[NW * 128, 128], [128, NWSB],
                                         [1, 42]]),
                                r2s[:])
                            # stash a_dst2 (col 41) for the L2 telescoping
                            nc.scalar.copy(
                                out=ad2own[:, 1 + s * NWSB:
                                           1 + (s + 1) * NWSB, :],
                                in_=r2s[:, :, 41:42])
                        else:
                            den = pw.tile([128, NWSB, 1], f32, tag="den2")
                            nc.scalar.activation(out=den[:],
                                                 in_=hsb[:, :, 40:41],
                                                 func=AF.Copy, bias=ACC_EPS)
                            rec = pw.tile([128, NWSB, 1], f32, tag="rec2")
                            nc.vector.reciprocal(
                                rec[:].rearrange("p a b -> p (a b)"),
                                den[:].rearrange("p a b -> p (a b)"))
                            o2 = pw.tile([128, NWSB, 40], f32, tag="o2")
                            nu = hsb[:, :, 0:40]
                            r_ = rec[:]
                            nc.vector.tensor_tensor(
                                out=o2[:],
                                in0=BC(nu, [nu.ap[0], [41, NWSB], [1, 40]]),
                                in1=BC(r_, [r_.ap[0], [1, NWSB], [0, 40]]),
                                op=OP.mult)
                            o2v = o2[:].rearrange("p a b -> p (a b)")
                            if not plan["skip_b2"]:
                                b2w = b2r[:]
                                nc.vector.tensor_tensor(
                                    out=o2v, in0=o2v,
                                    in1=BC(b2w, [b2w.ap[0], [0, NWSB],
                                                 [1, 40]]),
                                    op=OP.add)
                            nc.sync.dma_start(
                                bass.AP(out, (s * NWSB) * 128 * 40,
                                        [[40, 128], [128 * 40, NWSB],
                                         [1, 40]]),
                                o2[:])

            edge_pass(1, pb_ext=pb1, ph_ext=ph1)
            estack.close()

            # ---------- AllGather first; the at2 telescoping (local data
            # only) runs during the collective ----------
            nc.gpsimd.collective_compute(
                "AllGather", OP.bypass, ins=[agi2[:]], outs=[ago[:]],
                replica_groups=[list(range(8))])

            with (tc.tile_pool(name="pat2", bufs=2) as pat2,
                  tc.tile_pool(name="ppat2", bufs=2, space="PSUM") as ppat2,
                  tc.tile_pool(name="ppd2", bufs=1, space="PSUM") as ppd2):
                dall2 = pat2.tile([128, 2, NW, 1], f16, tag="dall")
                build_dall(ad2own, 1, dall2, ppd2)
                at_fill(dall2, at2_all, 1, pat2, ppat2)

            edge_pass(2)

    nc.finalize()
    return nc


def kernel(**inputs):
    per_core, plan = _host_prep(**inputs)
    if "nc" not in _CACHE:
        _CACHE["nc"] = _build_nc(plan)
    nc = _CACHE["nc"]
    from concourse.bass_utils import run_bass_kernel_spmd
    res = run_bass_kernel_spmd(nc, per_core, list(range(8)))
    full = np.concatenate([res.results[c]["out"] for c in range(8)], axis=0)
    return np.ascontiguousarray(full[:N]).astype(np.float32)

